# revision 24
# baseline (speedup 1.0000x reference)
"""Trainium2 Bass kernel for nn_Block_47193100648803.

Contract: kernel(**inputs) takes FULL unsharded inputs (numpy), returns the
FULL (N, O, T, V) output. Internally shards data-parallel over N across the
8 NeuronCores (one batch element per core, weights replicated).

v2 design (from trace analysis of v1 @ 709us):
- Channels on SBUF partitions (C=256 -> 2 half tiles), tokens on free axis.
- Phase 1 (frame domain, 3328 cols): LN1 (PE ones-matmul stats), q/k/v/qa
  projections in bf16. px = Wp@q + btp + x deferred to fill the PE gap
  during the softmax-heavy front.
- Front (global, group domain 9600 cols): qw softmax in 8 chunks (gather
  from DRAM bounce), pooled query pq via product+reduce, kp per-block on
  GpSimd, ka matmuls, kw softmax, pooled key pk. All Exp work stays in the
  natural_log_exp activation-table era (no table thrash).
- Backs (2 halves x 11 blocks, stage-major): att = Wtp@z + I@px_unf (PE
  identity-accumulated residual), LN folds with explicit mean subtraction
  (vector, bf16 2x mode), FFN + temporal conv. Scalar activation stream is
  batched per stage so table set switches drop from ~70 to ~10.
- Softmax biases bqa/bka are dropped entirely (softmax shift invariance);
  ln1 gamma/beta folded into the QKV weights/biases at setup.
"""

import os
import sys

import numpy as np

for _p in ("/opt/trn_rl_repo", "/root/.axon_site/_ro/trn_rl_repo"):
    if os.path.isdir(_p) and _p not in sys.path:
        sys.path.append(_p)

import concourse.bass as bass
import concourse.tile as tile
from concourse import bacc, bass_utils, mybir
from concourse.masks import make_identity

f32 = mybir.dt.float32
f32r = mybir.dt.float32r
bf16 = mybir.dt.bfloat16
AF = mybir.ActivationFunctionType
ALU = mybir.AluOpType
AX = mybir.AxisListType

# ---- problem constants (hardcoded per spec) ----
N_CORES = 8
C, T, V = 256, 128, 25
H = 8
W = 3
O = 256
L = W * V                 # 75
FT = T + 2                # 130 padded frames
F = FT * V                # 3250 real frame columns (zero pads at both ends)
F_PAD = 3328              # allocated frame columns
G = T                     # 128 groups per core
GL = G * L                # 9600 group-stage columns
SCALE = 1.0 / (32.0 ** 0.5)
EPS = 1e-5

FSUB = 416                # phase-1 matmul column tile
N_FSUB = F_PAD // FSUB    # 8
CH_G = 16                 # groups per softmax chunk (16*8 heads = 128 parts)
N_CH = G // CH_G          # 8

GH = 64                   # groups per back half
HALF_COLS = GH * L        # 4800
# back blocks within a half: 10x6 groups + 1x4 groups (450/300 cols)
BLOCKS = [(i * 6, 6) for i in range(10)] + [(60, 4)]
# global 6-group blocks for kp/ka (21x450 + 1x150)
KA_BLOCKS = [(i * 6, 6) for i in range(21)] + [(126, 2)]
C2B = 400                 # c2 output block (16 groups * 25)
N_C2B = HALF_COLS // 3 // C2B  # 1600/400 = 4


def _r(ap):
    return ap.bitcast(f32r)


def _view(t, offset, dims):
    """AP view on tile t: partition dim kept, free dims replaced."""
    return bass.AP(tensor=t.tensor, offset=t.offset + offset, ap=[t.ap[0]] + dims)


def unf(t, g0, gc):
    """Overlapping window view [128, gc, W, V] on a [128, F] frame tile."""
    return _view(t, g0 * V, [[V, gc], [V, W], [1, V]])


def bc_g(t, g0, gc):
    """Broadcast per-(c,g) [128, G] tile over L -> [128, gc, L] (step-0)."""
    return _view(t, g0, [[1, gc], [0, L]])


def build(nc):
    x_d = nc.dram_tensor("x", [C, T, V], f32, kind="ExternalInput").ap()
    wd = {}
    for nm in ["Wq", "Wk", "Wv", "Wt", "Wp", "W1", "W2", "c1_w"]:
        wd[nm] = nc.dram_tensor(nm, [C, C], f32, kind="ExternalInput").ap()
    wd["Wqa"] = nc.dram_tensor("Wqa", [C, H], f32, kind="ExternalInput").ap()
    wd["Wka"] = nc.dram_tensor("Wka", [C, H], f32, kind="ExternalInput").ap()
    wd["c2_w"] = nc.dram_tensor("c2_w", [W, C, O], f32, kind="ExternalInput").ap()
    bnames = ["ln1_g", "ln1_b", "bq", "bk", "bv", "bt", "bp", "ffn_g", "ffn_b",
              "b1", "b2", "tn_g", "tn_b", "c1_b", "c2_b"]
    for nm in bnames:
        wd[nm] = nc.dram_tensor(nm, [C], f32, kind="ExternalInput").ap()
    out_d = nc.dram_tensor("out", [O, T, V], f32, kind="ExternalOutput").ap()

    qa_d = nc.dram_tensor("qa_scr", [H, F_PAD], f32).ap()
    qw_d = nc.dram_tensor("qw_scr", [H, GL], bf16).ap()
    ka_d = nc.dram_tensor("ka_scr", [H, GL], f32).ap()
    kw_d = nc.dram_tensor("kw_scr", [H, GL], bf16).ap()
    row_d = nc.dram_tensor("row_scr", [6, C], f32).ap()

    with tile.TileContext(nc) as tc:
        with (
            tc.tile_pool(name="consts", bufs=1) as cp,
            tc.tile_pool(name="data", bufs=1) as dp,
        ):
            # ---------- input load first (weights go on other DMA queues) ----
            p1x_cm = tc.tile_pool(name="p1_x", bufs=1)
            p1x = p1x_cm.__enter__()
            x_f = [p1x.tile([128, F_PAD], f32, tag=f"x_f{hh}", name=f"x_f{hh}")
                   for hh in range(2)]
            for hh in range(2):
                nc.gpsimd.dma_start(out=_r(x_f[hh][:, V:F - V]),
                                    in_=_r(x_d[hh * 128:(hh + 1) * 128, :, :]))

            def load_bias_col(nm):
                t = cp.tile([128, 2], f32, tag=f"b_{nm}", name=f"b_{nm}")
                src = bass.AP(tensor=wd[nm].tensor, offset=wd[nm].offset,
                              ap=[[1, 128], [128, 2]])
                nc.scalar.dma_start(out=t, in_=src)
                return t

            bias = {nm: load_bias_col(nm) for nm in bnames}

            eps_t = cp.tile([128, 1], f32, tag="eps", name="eps_t")
            nc.vector.memset(eps_t, EPS)

            def fill_r(t, value):
                nc.scalar.activation(out=_r(t), in_=_r(t), func=AF.Copy,
                                     bias=float(value), scale=0.0)

            onesC = cp.tile([128, 128], f32, tag="onesC", name="onesC")
            fill_r(onesC, 1.0 / C)
            onesC_b = cp.tile([128, 128], bf16, tag="onesC_b", name="onesC_b")
            nc.scalar.activation(out=onesC_b, in_=onesC, func=AF.Copy)

            # ---- bf16 stationaries (ln1_g folded into Wq/Wk/Wv/Wqa rows) ----
            wqb = [cp.tile([128, C], bf16, tag=f"wqb{kh}", name=f"wqb{kh}") for kh in range(2)]
            wkb = [cp.tile([128, C], bf16, tag=f"wkb{kh}", name=f"wkb{kh}") for kh in range(2)]
            wvb = [cp.tile([128, C], bf16, tag=f"wvb{kh}", name=f"wvb{kh}") for kh in range(2)]
            wpb = [cp.tile([128, C], bf16, tag=f"wpb{kh}", name=f"wpb{kh}") for kh in range(2)]
            w2b = [cp.tile([128, C], bf16, tag=f"w2b{kh}", name=f"w2b{kh}") for kh in range(2)]
            w1g = [cp.tile([128, C], bf16, tag=f"w1g{kh}", name=f"w1g{kh}") for kh in range(2)]
            c1g = [cp.tile([128, C], bf16, tag=f"c1g{kh}", name=f"c1g{kh}") for kh in range(2)]
            wqab = [cp.tile([128, H], bf16, tag=f"wqab{kh}", name=f"wqab{kh}") for kh in range(2)]
            wkab = [cp.tile([128, H], bf16, tag=f"wkab{kh}", name=f"wkab{kh}") for kh in range(2)]
            c2t = []
            for w in range(W):
                c2t.append([cp.tile([128, O], bf16, tag=f"w_c2_{w}{kh}", name=f"w_c2_{w}{kh}")
                            for kh in range(2)])
            wtpb = [cp.tile([128, C], bf16, tag=f"wtpb{kh}", name=f"wtpb{kh}") for kh in range(2)]
            ident_b = cp.tile([128, 128], bf16, tag="ident_b", name="ident_b")

            # ---------- setup-scoped: Wtp = Wt@Wp, bias rows, c2 cast ----------
            with (
                tc.tile_pool(name="setup_sb", bufs=1) as sp,
                tc.tile_pool(name="setup_ps", bufs=2, space="PSUM") as spp,
            ):
                # raw f32 weights (setup-scoped; freed before phase 1)
                wt = {}
                for i, nm in enumerate(["Wq", "Wk", "Wv", "Wp", "W1", "W2", "c1_w"]):
                    wt[nm] = [sp.tile([128, C], f32, tag=f"w_{nm}{kh}", name=f"w_{nm}{kh}")
                              for kh in range(2)]
                    eng = nc.scalar if i % 2 == 0 else nc.sync
                    for kh in range(2):
                        eng.dma_start(out=_r(wt[nm][kh]),
                                      in_=_r(wd[nm][kh * 128:(kh + 1) * 128, :]))
                for nm in ["Wqa", "Wka"]:
                    wt[nm] = [sp.tile([128, H], f32, tag=f"w_{nm}{kh}", name=f"w_{nm}{kh}")
                              for kh in range(2)]
                    for kh in range(2):
                        nc.scalar.dma_start(out=_r(wt[nm][kh]),
                                            in_=_r(wd[nm][kh * 128:(kh + 1) * 128, :]))
                # ln1_g folded into Wq/Wk/Wv rows; wqab = g * (Wq @ Wqa)
                # below — the reference pools attention logits from q
                # (= nx@Wq + bq); the bq/bqa shifts cancel under softmax.
                for kh in range(2):
                    g_col = bias["ln1_g"][:, kh:kh + 1]
                    nc.vector.tensor_scalar_mul(wqb[kh], wt["Wq"][kh], g_col)
                    nc.vector.tensor_scalar_mul(wkb[kh], wt["Wk"][kh], g_col)
                    nc.vector.tensor_scalar_mul(wvb[kh], wt["Wv"][kh], g_col)
                    nc.vector.tensor_scalar_mul(w1g[kh], wt["W1"][kh],
                                                bias["ffn_g"][:, kh:kh + 1])
                    nc.vector.tensor_scalar_mul(c1g[kh], wt["c1_w"][kh],
                                                bias["tn_g"][:, kh:kh + 1])
                    nc.scalar.activation(out=wpb[kh], in_=wt["Wp"][kh], func=AF.Copy)
                    nc.scalar.activation(out=w2b[kh], in_=wt["W2"][kh], func=AF.Copy)
                    nc.vector.tensor_scalar_mul(wkab[kh], wt["Wka"][kh], 1.0)

                c2f = sp.tile([128, O], f32, tag="c2f", bufs=2, name="c2f")
                for w in range(W):
                    for kh in range(2):
                        c2f_ = sp.tile([128, O], f32, tag="c2f", bufs=2, name="c2f_")
                        nc.sync.dma_start(out=c2f_,
                                          in_=wd["c2_w"][w, kh * 128:(kh + 1) * 128, :])
                        nc.vector.tensor_copy(c2t[w][kh], c2f_)

                wtw = [sp.tile([128, C], f32, tag=f"wt{kh}", name=f"wtw{kh}")
                       for kh in range(2)]
                for kh in range(2):
                    nc.sync.dma_start(out=wtw[kh],
                                      in_=wd["Wt"][kh * 128:(kh + 1) * 128, :])
                ident = sp.tile([128, 128], f32, tag="ident", name="ident")
                make_identity(nc, ident)
                nc.scalar.activation(out=ident_b, in_=ident, func=AF.Copy)

                for kh in range(2):
                    pacc = spp.tile([128, C], f32, tag="wtp_acc", name="pacc")
                    pqa_w = spp.tile([128, H], f32, tag="qae_acc", name="pqa_w")
                    for mh in range(2):
                        ptr = spp.tile([128, 128], f32, tag="tr", name="ptr")
                        nc.tensor.transpose(ptr, wtw[kh][:, mh * 128:(mh + 1) * 128], ident)
                        a_t = sp.tile([128, 128], f32, tag="a_t", name="a_t")
                        nc.scalar.activation(out=_r(a_t), in_=ptr, func=AF.Copy)
                        nc.tensor.matmul(pacc, _r(a_t), _r(wt["Wp"][mh]),
                                         start=(mh == 0), stop=(mh == 1))
                        # Wqa_eff[kh] = sum_m Wq[kh rows, m]^T.T @ Wqa[m]
                        ptr2 = spp.tile([128, 128], f32, tag="tr", name="ptr2")
                        nc.tensor.transpose(ptr2, wt["Wq"][kh][:, mh * 128:(mh + 1) * 128], ident)
                        a_t2 = sp.tile([128, 128], f32, tag="a_t", name="a_t2")
                        nc.scalar.activation(out=_r(a_t2), in_=ptr2, func=AF.Copy)
                        nc.tensor.matmul(pqa_w, _r(a_t2), _r(wt["Wqa"][mh]),
                                         start=(mh == 0), stop=(mh == 1))
                    nc.scalar.activation(out=wtpb[kh], in_=pacc, func=AF.Copy)
                    nc.vector.tensor_scalar_mul(wqab[kh], pqa_w,
                                                bias["ln1_g"][:, kh:kh + 1])

                def colvec(nm, kh):
                    t = sp.tile([128, 1], f32, tag="cv", bufs=4, name=f"cv_{nm}{kh}")
                    src = bass.AP(tensor=wd[nm].tensor, offset=wd[nm].offset + kh * 128,
                                  ap=[[1, 128], [128, 1]])
                    nc.sync.dma_start(out=_r(t), in_=_r(src))
                    return t

                def rowvec(nm):
                    t = sp.tile([1, C], f32, tag="rv", bufs=4, name=f"rv_{nm}")
                    nc.sync.dma_start(out=t, in_=wd[nm])
                    return t

                # rows: btp = bt@Wp + bp; B1 = ffn_b@W1 + b1; Bc1 = tn_b@c1_w
                #       + c1_b; bq' = ln1_b@Wq + bq; similarly bk', bv'
                for i, (bnm, wmat, addnm) in enumerate([
                    ("bt", wt["Wp"], "bp"),
                    ("ffn_b", wt["W1"], "b1"),
                    ("tn_b", wt["c1_w"], "c1_b"),
                    ("ln1_b", wt["Wq"], "bq"),
                    ("ln1_b", wt["Wk"], "bk"),
                    ("ln1_b", wt["Wv"], "bv"),
                ]):
                    pr = spp.tile([1, C], f32, tag="rowacc", name="pr")
                    for kh in range(2):
                        nc.tensor.matmul(pr, _r(colvec(bnm, kh)), _r(wmat[kh]),
                                         start=(kh == 0), stop=(kh == 1))
                    row_i = sp.tile([1, C], f32, tag="row_i", bufs=3, name=f"row_i{i}")
                    nc.vector.tensor_add(row_i, pr, rowvec(addnm))
                    nc.sync.dma_start(out=row_d[i:i + 1, :], in_=row_i)

            # bounce bias rows back into per-partition [128, 2] layout
            btp_t = cp.tile([128, 2], f32, tag="btp", name="btp_t")
            B1_t = cp.tile([128, 2], f32, tag="B1", name="B1_t")
            Bc1_t = cp.tile([128, 2], f32, tag="Bc1", name="Bc1_t")
            bq_t = cp.tile([128, 2], f32, tag="bqf", name="bq_t")
            bk_t = cp.tile([128, 2], f32, tag="bkf", name="bk_t")
            bv_t = cp.tile([128, 2], f32, tag="bvf", name="bv_t")
            for i, t in enumerate([btp_t, B1_t, Bc1_t, bq_t, bk_t, bv_t]):
                for kh in range(2):
                    src = bass.AP(tensor=row_d.tensor,
                                  offset=row_d.offset + i * C + kh * 128,
                                  ap=[[1, 128], [128, 1]])
                    nc.sync.dma_start(out=t[:, kh:kh + 1], in_=src)

            # ---------- persistent activations ----------
            q_f = [dp.tile([128, F_PAD], bf16, tag=f"q_f{hh}", name=f"q_f{hh}") for hh in range(2)]
            k_f = [dp.tile([128, F_PAD], bf16, tag=f"k_f{hh}", name=f"k_f{hh}") for hh in range(2)]
            v_f = [dp.tile([128, F_PAD], bf16, tag=f"v_f{hh}", name=f"v_f{hh}") for hh in range(2)]
            px_f = [dp.tile([128, F_PAD], bf16, tag=f"px_f{hh}", name=f"px_f{hh}") for hh in range(2)]
            pq_b = [dp.tile([128, G], bf16, tag=f"pqb{hh}", name=f"pqb{hh}") for hh in range(2)]
            pk_b = [dp.tile([128, G], bf16, tag=f"pkb{hh}", name=f"pkb{hh}") for hh in range(2)]

            def softmax_chunk(src_gather_ap, dst_dram, g0, pool, tagp):
                """Softmax over L per (group, head) in [128 = 16g x 8h, L]
                layout; writes normalized bf16 weights to dst_dram."""
                ag = pool.tile([128, L], f32, tag="sm_ag", bufs=4, name=f"ag_{tagp}")
                nc.gpsimd.dma_start(out=ag, in_=src_gather_ap)
                mx = pool.tile([128, 1], f32, tag="sm_mx", bufs=4, name=f"mx_{tagp}")
                nc.vector.reduce_max(mx, ag, axis=AX.X)
                e = pool.tile([128, L], f32, tag="sm_e", bufs=4, name=f"e_{tagp}")
                nc.vector.tensor_scalar_sub(e, ag, mx[:, 0:1])
                nc.scalar.activation(out=e, in_=e, func=AF.Exp, scale=SCALE)
                sm = pool.tile([128, 1], f32, tag="sm_s", bufs=4, name=f"sm_{tagp}")
                nc.vector.reduce_sum(sm, e, axis=AX.X)
                rs = pool.tile([128, 1], f32, tag="sm_rs", bufs=4, name=f"rs_{tagp}")
                nc.vector.reciprocal(rs, sm)
                wgn = pool.tile([128, L], bf16, tag="sm_w", bufs=4, name=f"wgn_{tagp}")
                nc.vector.tensor_scalar_mul(wgn, e, rs[:, 0:1])
                dst = bass.AP(tensor=dst_dram.tensor,
                              offset=dst_dram.offset + g0 * L,
                              ap=[[L, CH_G], [GL, H], [1, L]])
                nc.gpsimd.dma_start(out=dst, in_=wgn)

            # ================= phase 1 + front =================
            fp_cm = tc.tile_pool(name="front_sb", bufs=1)
            fp = fp_cm.__enter__()
            qw_bc = [fp.tile([128, GL], bf16, tag="bc", bufs=2, name=f"qwbc{hh}")
                     for hh in range(2)]

            p1_cm = tc.tile_pool(name="p1_sb", bufs=2)
            pp1_cm = tc.tile_pool(name="p1_ps", bufs=1, space="PSUM")
            pp1m_cm = tc.tile_pool(name="p1_mm", bufs=4, space="PSUM")
            p1 = p1_cm.__enter__()
            pp1 = pp1_cm.__enter__()
            pp1m = pp1m_cm.__enter__()
            if True:
                for hh in range(2):
                    fill_r(x_f[hh][:, 0:V], 0.0)
                    fill_r(x_f[hh][:, F - V:F_PAD], 0.0)

                def phase1_fsub(s):
                    sl = slice(s * FSUB, (s + 1) * FSUB)
                    x2 = [p1.tile([128, FSUB], f32, tag=f"x2_{hh}", name=f"x2_{hh}")
                          for hh in range(2)]
                    for hh in range(2):
                        nc.vector.scalar_tensor_tensor(
                            out=_r(x2[hh]), in0=x_f[hh][:, sl], scalar=1.0,
                            in1=x_f[hh][:, sl], op0=ALU.mult, op1=ALU.mult)
                    pmean = pp1.tile([128, FSUB], f32, tag="pmean", name="pmean")
                    pmsq = pp1.tile([128, FSUB], f32, tag="pmsq", name="pmsq")
                    for hh in range(2):
                        nc.tensor.matmul(pmean, _r(onesC), _r(x_f[hh][:, sl]),
                                         start=(hh == 0), stop=(hh == 1))
                    for hh in range(2):
                        nc.tensor.matmul(pmsq, _r(onesC), _r(x2[hh]),
                                         start=(hh == 0), stop=(hh == 1))
                    m2 = p1.tile([128, FSUB], f32, tag="m2", name="m2")
                    nc.scalar.activation(out=m2, in_=pmean, func=AF.Square)
                    var = p1.tile([128, FSUB], f32, tag="var", name="var")
                    nc.vector.tensor_sub(var, pmsq, m2)
                    lnv = p1.tile([128, FSUB], f32, tag="sd", name="lnv")
                    nc.scalar.activation(out=lnv, in_=var, func=AF.Ln, bias=eps_t)
                    rstd = p1.tile([128, FSUB], f32, tag="rstd", name="rstd")
                    nc.scalar.activation(out=rstd, in_=lnv, func=AF.Exp, scale=-0.5)
                    nx = []
                    for hh in range(2):
                        xc = p1.tile([128, FSUB], f32, tag=f"xc{hh}", name=f"xc{hh}")
                        nc.vector.tensor_sub(xc, x_f[hh][:, sl], pmean)
                        nxh = p1.tile([128, FSUB], bf16, tag=f"nx{hh}", name=f"nx{hh}")
                        nc.vector.tensor_mul(nxh, xc, rstd)
                        nx.append(nxh)
                    for nm_w, b_t, dst, eng in [(wqb, bq_t, q_f, "s"),
                                                (wkb, bk_t, k_f, "s"),
                                                (wvb, bv_t, v_f, "v")]:
                        for mh in range(2):
                            pm_ = pp1m.tile([128, FSUB], f32, tag="mm", name="pm_")
                            for kh in range(2):
                                nc.tensor.matmul(pm_,
                                                 nm_w[kh][:, mh * 128:(mh + 1) * 128],
                                                 nx[kh], start=(kh == 0), stop=(kh == 1))
                            if eng == "s":
                                nc.scalar.activation(out=dst[mh][:, sl], in_=pm_,
                                                     func=AF.Identity,
                                                     bias=b_t[:, mh:mh + 1])
                            else:
                                nc.vector.tensor_scalar_add(dst[mh][:, sl], pm_,
                                                            b_t[:, mh:mh + 1])
                    pqa = pp1.tile([H, FSUB], f32, tag="pqa", name="pqa")
                    for kh in range(2):
                        nc.tensor.matmul(pqa, wqab[kh], nx[kh],
                                         start=(kh == 0), stop=(kh == 1))
                    qa_s = p1.tile([H, FSUB], f32, tag="qa_s", bufs=3, name="qa_s")
                    nc.vector.tensor_copy(qa_s, pqa)
                    nc.sync.dma_start(out=qa_d[:, sl], in_=qa_s)

                def qw_chunk(cc):
                    g0 = cc * CH_G
                    src = bass.AP(tensor=qa_d.tensor, offset=qa_d.offset + g0 * V,
                                  ap=[[V, CH_G], [F_PAD, H], [1, L]])
                    softmax_chunk(src, qw_d, g0, p1, "q")
                    # broadcast this chunk of qw into group-stage layout
                    for hh in range(2):
                        src_b = bass.AP(
                            tensor=qw_d.tensor,
                            offset=qw_d.offset + (hh * 4) * GL + g0 * L,
                            ap=[[GL, 4], [0, 32], [1, CH_G * L]])
                        nc.sync.dma_start(out=qw_bc[hh][:, g0 * L:(g0 + CH_G) * L],
                                          in_=src_b)

                # qw chunk cc needs qa cols up to cc*400+450 -> ready after
                # FSUB ceil((cc*400+450)/416); interleave emission
                for s in range(N_FSUB):
                    phase1_fsub(s)
                    if s >= 2:
                        qw_chunk(s - 2)
                for cc in range(N_CH - 2, N_CH):
                    qw_chunk(cc)

                # ---- pooled query (global product + reduce) ----
                pq_t = [dp.tile([128, G], f32, tag=f"pq{hh}", name=f"pq{hh}")
                        for hh in range(2)]
                for hh in range(2):
                    prod = fp.tile([128, GL], bf16, tag="prod", bufs=1,
                                   name=f"prodq{hh}")
                    nc.vector.scalar_tensor_tensor(
                        out=prod, in0=unf(q_f[hh], 0, G), scalar=1.0,
                        in1=qw_bc[hh], op0=ALU.mult, op1=ALU.mult)
                    nc.vector.reduce_sum(pq_t[hh], _view(prod, 0, [[L, G], [1, L]]),
                                         axis=AX.X)
                    nc.vector.tensor_copy(pq_b[hh], pq_t[hh])

                # ---- px = Wp@q + btp + x (deferred: fills PE gap here) ----
                for s in range(N_FSUB):
                    sl = slice(s * FSUB, (s + 1) * FSUB)
                    for mh in range(2):
                        pp_ = pp1m.tile([128, FSUB], f32, tag="mm", name="pp_")
                        for kh in range(2):
                            nc.tensor.matmul(pp_, wpb[kh][:, mh * 128:(mh + 1) * 128],
                                             q_f[kh][:, sl], start=(kh == 0),
                                             stop=(kh == 1))
                        nc.vector.scalar_tensor_tensor(
                            out=px_f[mh][:, sl], in0=pp_, scalar=btp_t[:, mh:mh + 1],
                            in1=x_f[mh][:, sl], op0=ALU.add, op1=ALU.add)

                pp1m_cm.__exit__(None, None, None)
                pp1_cm.__exit__(None, None, None)

                # ---- kp (gpsimd) + ka matmuls + kw softmax ----
                with tc.tile_pool(name="ka_ps", bufs=2, space="PSUM") as kap:
                    for (ga, gc) in KA_BLOCKS:
                        cw = gc * L
                        col0 = ga * L
                        kp_blk = []
                        for hh in range(2):
                            kpb = fp.tile([128, 6 * L], bf16, tag="kp", bufs=4,
                                          name=f"kp{hh}")
                            nc.gpsimd.tensor_tensor(
                                out=_view(kpb, 0, [[L, gc], [1, L]]),
                                in0=unf(k_f[hh], ga, gc),
                                in1=bc_g(pq_b[hh], ga, gc), op=ALU.mult)
                            kp_blk.append(kpb)
                        pka = kap.tile([H, 6 * L], f32, tag="ka", name="pka")
                        for hh in range(2):
                            nc.tensor.matmul(pka[:, 0:cw], wkab[hh],
                                             kp_blk[hh][:, 0:cw],
                                             start=(hh == 0), stop=(hh == 1))
                        ka_s = fp.tile([H, 6 * L], f32, tag="ka_s", bufs=3,
                                       name="ka_s")
                        nc.scalar.activation(out=ka_s[:, 0:cw],
                                             in_=pka[:, 0:cw], func=AF.Copy)
                        nc.scalar.dma_start(out=ka_d[:, col0:col0 + cw],
                                            in_=ka_s[:, 0:cw])

                    kw_bc = [fp.tile([128, GL], bf16, tag="bc", bufs=2,
                                     name=f"kwbc{hh}") for hh in range(2)]
                    for cc in range(N_CH):
                        g0 = cc * CH_G
                        src = bass.AP(tensor=ka_d.tensor,
                                      offset=ka_d.offset + g0 * L,
                                      ap=[[L, CH_G], [GL, H], [1, L]])
                        softmax_chunk(src, kw_d, g0, p1, "k")
                        for hh in range(2):
                            src_b = bass.AP(
                                tensor=kw_d.tensor,
                                offset=kw_d.offset + (hh * 4) * GL + g0 * L,
                                ap=[[GL, 4], [0, 32], [1, CH_G * L]])
                            nc.sync.dma_start(
                                out=kw_bc[hh][:, g0 * L:(g0 + CH_G) * L],
                                in_=src_b)

                    # ---- pooled key pk ----
                    pk_t = [dp.tile([128, G], f32, tag=f"pk{hh}", name=f"pk{hh}")
                            for hh in range(2)]
                    for hh in range(2):
                        prod = fp.tile([128, GL], bf16, tag="prod", bufs=1,
                                       name=f"prodk{hh}")
                        nc.vector.scalar_tensor_tensor(
                            out=prod, in0=unf(k_f[hh], 0, G), scalar=1.0,
                            in1=kw_bc[hh], op0=ALU.mult, op1=ALU.mult)
                        nc.vector.reduce_sum(pk_t[hh],
                                             _view(prod, 0, [[L, G], [1, L]]),
                                             axis=AX.X)
                        nc.vector.tensor_copy(pk_b[hh], pk_t[hh])

            p1_cm.__exit__(None, None, None)
            fp_cm.__exit__(None, None, None)
            p1x_cm.__exit__(None, None, None)

            # ================= backs: 2 halves x 11 blocks =================
            with (
                tc.tile_pool(name="bk_sb", bufs=1) as bp,
                tc.tile_pool(name="bk_sm", bufs=1) as bs,
                tc.tile_pool(name="bk_ps", bufs=1, space="PSUM") as bps,
            ):
                def big(name):
                    return bp.tile([128, HALF_COLS], bf16, tag="big", bufs=8,
                                   name=name)

                def small(tag, dt=bf16):
                    return bs.tile([128, 6 * L], dt, tag=tag, bufs=4, name=tag)

                def layer_mm(pm, wpair, rhs_pair, cols, extra=None):
                    """pm[mh] = sum_kh wpair[kh][:,mh]^T @ rhs_pair[kh][:,cols]
                    (+ I @ extra[mh][:,cols])."""
                    for mh in range(2):
                        for kh in range(2):
                            nc.tensor.matmul(pm[mh], wpair[kh][:, mh * 128:(mh + 1) * 128],
                                             rhs_pair[kh][:, cols],
                                             start=(kh == 0),
                                             stop=(kh == 1) and extra is None)
                        if extra is not None:
                            nc.tensor.matmul(pm[mh], ident_b, extra[mh][:, cols],
                                             start=False, stop=True)

                def stats_mm(ps, src0, src1, cols):
                    nc.tensor.matmul(ps, onesC_b, src0[:, cols], start=True, stop=False)
                    nc.tensor.matmul(ps, onesC_b, src1[:, cols], start=False, stop=True)

                for ih, g0h in enumerate([0, GH]):
                    # one "big" tag, bufs=8: call i+8 reuses call i's buffer.
                    # Allocation order guarantees the prior tenant's last read
                    # stage strictly precedes the new tenant's first write
                    # stage (pool reuse is tile-granular, not subtile).
                    z_t = [big(f"z{ih}{hh}") for hh in range(2)]
                    px_u = [big(f"px{ih}{mh}") for mh in range(2)]
                    xr1 = [big(f"xr{ih}{mh}") for mh in range(2)]
                    att_b = [big(f"att{ih}{mh}") for mh in range(2)]
                    g1 = [big(f"g1{ih}{mh}") for mh in range(2)]
                    y_b = [big(f"y{ih}{mh}") for mh in range(2)]
                    yr = [big(f"yr{ih}{mh}") for mh in range(2)]
                    h_t = [big(f"h{ih}{mh}") for mh in range(2)]

                    # --- z = v * pk (gpsimd, per block) ---
                    for hh in range(2):
                        for (ga, gc) in BLOCKS:
                            nc.gpsimd.tensor_tensor(
                                out=_view(z_t[hh], ga * L, [[L, gc], [1, L]]),
                                in0=unf(v_f[hh], g0h + ga, gc),
                                in1=bc_g(pk_b[hh], g0h + ga, gc), op=ALU.mult)
                    # --- px_unf via SBUF->SBUF window DMA (3 slabs) ---
                    for mh in range(2):
                        for (ga, gc) in [(0, 22), (22, 21), (43, 21)]:
                            nc.sync.dma_start(
                                out=px_u[mh][:, ga * L:(ga + gc) * L],
                                in_=unf(px_f[mh], g0h + ga, gc))

                    def fused_ln_block(pm, dst_pair, xr_pair, b2col, cols, cw):
                        """Evict pm -> dst (+optional bias), square, LN stats,
                        normalize: xr = (dst - mean) * rstd. Per-block."""
                        sq = small("sq")
                        for mh in range(2):
                            if b2col is None:
                                nc.vector.tensor_copy(dst_pair[mh][:, cols], pm[mh])
                            else:
                                nc.vector.tensor_scalar_add(dst_pair[mh][:, cols],
                                                            pm[mh], b2col[:, mh:mh + 1])
                        sq2 = small("sq2")
                        for mh, sqt in ((0, sq), (1, sq2)):
                            if b2col is None:
                                nc.scalar.activation(out=sqt[:, 0:cw], in_=pm[mh],
                                                     func=AF.Square)
                            else:
                                nc.scalar.activation(out=sqt[:, 0:cw], in_=pm[mh],
                                                     func=AF.Square,
                                                     bias=b2col[:, mh:mh + 1])
                        mps = bps.tile([128, 6 * L], f32, tag="mean", bufs=2,
                                       name="mps")[:, 0:cw]
                        sps = bps.tile([128, 6 * L], f32, tag="msq", bufs=2,
                                       name="sps")[:, 0:cw]
                        stats_mm(mps, dst_pair[0], dst_pair[1], cols)
                        nc.tensor.matmul(sps, onesC_b, sq[:, 0:cw], start=True, stop=False)
                        nc.tensor.matmul(sps, onesC_b, sq2[:, 0:cw], start=False, stop=True)
                        meanb = small("meanb")
                        nc.scalar.activation(out=meanb[:, 0:cw], in_=mps, func=AF.Copy)
                        m2 = small("m2")
                        nc.vector.scalar_tensor_tensor(
                            out=m2[:, 0:cw], in0=meanb[:, 0:cw], scalar=1.0,
                            in1=meanb[:, 0:cw], op0=ALU.mult, op1=ALU.mult)
                        var = small("var", f32)
                        nc.vector.scalar_tensor_tensor(
                            out=var[:, 0:cw], in0=sps, scalar=1.0, in1=m2[:, 0:cw],
                            op0=ALU.mult, op1=ALU.subtract)
                        lnv = small("lnv", f32)
                        nc.scalar.activation(out=lnv[:, 0:cw], in_=var[:, 0:cw],
                                             func=AF.Ln, bias=eps_t)
                        rstd = small("rstd")
                        nc.scalar.activation(out=rstd[:, 0:cw], in_=lnv[:, 0:cw],
                                             func=AF.Exp, scale=-0.5)
                        for mh in range(2):
                            xc = small(f"xc{mh}")
                            nc.vector.tensor_sub(xc[:, 0:cw], dst_pair[mh][:, cols],
                                                 meanb[:, 0:cw])
                            nc.vector.tensor_mul(xr_pair[mh][:, cols], xc[:, 0:cw],
                                                 rstd[:, 0:cw])

                    # --- stage P: att = Wtp@z + I@px, LN fold -> xr1 ---
                    for (ga, gc) in BLOCKS:
                        cols = slice(ga * L, (ga + gc) * L)
                        cw = gc * L
                        pm = [bps.tile([128, 6 * L], f32, tag="mm", bufs=2,
                                       name=f"pmP{mh}")[:, 0:cw] for mh in range(2)]
                        layer_mm(pm, wtpb, z_t, cols, extra=px_u)
                        fused_ln_block(pm, att_b, xr1, None, cols, cw)

                    # --- W1 -> gelu ---
                    for (ga, gc) in BLOCKS:
                        cols = slice(ga * L, (ga + gc) * L)
                        cw = gc * L
                        pm = [bps.tile([128, 6 * L], f32, tag="mm", bufs=2,
                                       name=f"pm1{mh}")[:, 0:cw] for mh in range(2)]
                        layer_mm(pm, w1g, xr1, cols)
                        for mh in range(2):
                            nc.scalar.activation(out=g1[mh][:, cols], in_=pm[mh],
                                                 func=AF.Gelu,
                                                 bias=B1_t[:, mh:mh + 1])

                    # --- W2 + I@att -> y, LN fold -> yr ---
                    for (ga, gc) in BLOCKS:
                        cols = slice(ga * L, (ga + gc) * L)
                        cw = gc * L
                        pm = [bps.tile([128, 6 * L], f32, tag="mm", bufs=2,
                                       name=f"pm2{mh}")[:, 0:cw] for mh in range(2)]
                        layer_mm(pm, w2b, g1, cols, extra=att_b)
                        fused_ln_block(pm, y_b, yr, bias["b2"], cols, cw)

                    # --- c1 -> gelu, h in w-major layout [128, W, GH*V] ---
                    for (ga, gc) in BLOCKS:
                        cols = slice(ga * L, (ga + gc) * L)
                        pm = [bps.tile([128, 6 * L], f32, tag="mm", bufs=2,
                                       name=f"pm3{mh}")[:, 0:gc * L] for mh in range(2)]
                        layer_mm(pm, c1g, yr, cols)
                        for mh in range(2):
                            dst = _view(h_t[mh], ga * V,
                                        [[V, gc], [GH * V, W], [1, V]])
                            src = _view(pm[mh], 0, [[L, gc], [V, W], [1, V]])
                            nc.scalar.activation(out=dst, in_=src, func=AF.Gelu,
                                                 bias=Bc1_t[:, mh:mh + 1])

                    # --- c2: contract (w, kh) -> out [O, GH*V] ---
                    for mh in range(2):
                        for cb in range(N_C2B):
                            po = bps.tile([128, C2B], f32, tag="po", bufs=2,
                                          name="po")
                            first = True
                            for w in range(W):
                                for kh in range(2):
                                    c0 = w * GH * V + cb * C2B
                                    nc.tensor.matmul(
                                        po, c2t[w][kh][:, mh * 128:(mh + 1) * 128],
                                        h_t[kh][:, c0:c0 + C2B], start=first,
                                        stop=(w == W - 1 and kh == 1))
                                    first = False
                            os_ = bs.tile([128, C2B], f32, tag="os", bufs=3,
                                          name="os_")
                            nc.scalar.activation(out=os_, in_=po, func=AF.Identity,
                                                 bias=bias["c2_b"][:, mh:mh + 1])
                            nc.sync.dma_start(
                                out=out_d[mh * 128:(mh + 1) * 128,
                                          g0h + cb * 16:g0h + (cb + 1) * 16, :],
                                in_=os_)
    return nc


_CACHE = {}


def _get_compiled():
    if "nc" not in _CACHE:
        nc = bacc.Bacc("TRN2", target_bir_lowering=False, debug=False)
        build(nc)
        nc.compile()
        _CACHE["nc"] = nc
    return _CACHE["nc"]


def kernel(**inputs):
    nc = _get_compiled()
    x = np.asarray(inputs["x"], dtype=np.float32)
    n = x.shape[0]
    names = ["Wq", "Wk", "Wv", "Wt", "Wp", "W1", "W2", "c1_w", "Wqa", "Wka",
             "c2_w", "ln1_g", "ln1_b", "bq", "bk", "bv", "bt", "bp", "ffn_g",
             "ffn_b", "b1", "b2", "tn_g", "tn_b", "c1_b", "c2_b"]
    shared = {nm: np.asarray(inputs[nm], dtype=np.float32) for nm in names}
    in_maps = [{"x": x[i], **shared} for i in range(n)]
    res = bass_utils.run_bass_kernel_spmd(nc, in_maps, core_ids=list(range(n)))
    return np.stack([res.results[i]["out"] for i in range(n)], axis=0)


if __name__ == "__main__":
    nc = bacc.Bacc("TRN2", target_bir_lowering=False, debug=False)
    build(nc)
    nc.compile()
    print("build+compile OK")


# revision 27
# speedup vs baseline: 1.1555x; 1.1555x over previous
"""Trainium2 Bass kernel for nn_Block_47193100648803.

Contract: kernel(**inputs) takes FULL unsharded inputs (numpy), returns the
FULL (N, O, T, V) output. Internally shards data-parallel over N across the
8 NeuronCores (one batch element per core, weights replicated).

v2 design (from trace analysis of v1 @ 709us):
- Channels on SBUF partitions (C=256 -> 2 half tiles), tokens on free axis.
- Phase 1 (frame domain, 3328 cols): LN1 (PE ones-matmul stats), q/k/v/qa
  projections in bf16. px = Wp@q + btp + x deferred to fill the PE gap
  during the softmax-heavy front.
- Front (global, group domain 9600 cols): qw softmax in 8 chunks (gather
  from DRAM bounce), pooled query pq via product+reduce, kp per-block on
  GpSimd, ka matmuls, kw softmax, pooled key pk. All Exp work stays in the
  natural_log_exp activation-table era (no table thrash).
- Backs (2 halves x 11 blocks, stage-major): att = Wtp@z + I@px_unf (PE
  identity-accumulated residual), LN folds with explicit mean subtraction
  (vector, bf16 2x mode), FFN + temporal conv. Scalar activation stream is
  batched per stage so table set switches drop from ~70 to ~10.
- Softmax biases bqa/bka are dropped entirely (softmax shift invariance);
  ln1 gamma/beta folded into the QKV weights/biases at setup.
"""

import os
import sys

import numpy as np

for _p in ("/opt/trn_rl_repo", "/root/.axon_site/_ro/trn_rl_repo"):
    if os.path.isdir(_p) and _p not in sys.path:
        sys.path.append(_p)

import concourse.bass as bass
import concourse.tile as tile
from concourse import bacc, bass_utils, mybir
from concourse.masks import make_identity

f32 = mybir.dt.float32
f32r = mybir.dt.float32r
bf16 = mybir.dt.bfloat16
AF = mybir.ActivationFunctionType
ALU = mybir.AluOpType
AX = mybir.AxisListType

# ---- problem constants (hardcoded per spec) ----
N_CORES = 8
C, T, V = 256, 128, 25
H = 8
W = 3
O = 256
L = W * V                 # 75
FT = T + 2                # 130 padded frames
F = FT * V                # 3250 real frame columns (zero pads at both ends)
F_PAD = 3328              # allocated frame columns
G = T                     # 128 groups per core
GL = G * L                # 9600 group-stage columns
SCALE = 1.0 / (32.0 ** 0.5)
EPS = 1e-5

FSUB = 416                # phase-1 matmul column tile
N_FSUB = F_PAD // FSUB    # 8
CH_G = 16                 # groups per softmax chunk (16*8 heads = 128 parts)
N_CH = G // CH_G          # 8

GH = 64                   # groups per back half
HALF_COLS = GH * L        # 4800
# back blocks within a half: 10x6 groups + 1x4 groups (450/300 cols)
BLOCKS = [(i * 6, 6) for i in range(10)] + [(60, 4)]
# global 6-group blocks for kp/ka (21x450 + 1x150)
KA_BLOCKS = [(i * 6, 6) for i in range(21)] + [(126, 2)]
C2B = 400                 # c2 output block (16 groups * 25)
N_C2B = HALF_COLS // 3 // C2B  # 1600/400 = 4


def _r(ap):
    return ap.bitcast(f32r)


def _view(t, offset, dims):
    """AP view on tile t: partition dim kept, free dims replaced."""
    return bass.AP(tensor=t.tensor, offset=t.offset + offset, ap=[t.ap[0]] + dims)


def unf(t, g0, gc):
    """Overlapping window view [128, gc, W, V] on a [128, F] frame tile."""
    return _view(t, g0 * V, [[V, gc], [V, W], [1, V]])


def bc_g(t, g0, gc):
    """Broadcast per-(c,g) [128, G] tile over L -> [128, gc, L] (step-0)."""
    return _view(t, g0, [[1, gc], [0, L]])


def build(nc):
    x_d = nc.dram_tensor("x", [C, T, V], f32, kind="ExternalInput").ap()
    wd = {}
    for nm in ["Wq", "Wk", "Wv", "Wt", "Wp", "W1", "W2", "c1_w"]:
        wd[nm] = nc.dram_tensor(nm, [C, C], f32, kind="ExternalInput").ap()
    wd["Wqa"] = nc.dram_tensor("Wqa", [C, H], f32, kind="ExternalInput").ap()
    wd["Wka"] = nc.dram_tensor("Wka", [C, H], f32, kind="ExternalInput").ap()
    wd["c2_w"] = nc.dram_tensor("c2_w", [W, C, O], f32, kind="ExternalInput").ap()
    bnames = ["ln1_g", "ln1_b", "bq", "bk", "bv", "bt", "bp", "ffn_g", "ffn_b",
              "b1", "b2", "tn_g", "tn_b", "c1_b", "c2_b"]
    for nm in bnames:
        wd[nm] = nc.dram_tensor(nm, [C], f32, kind="ExternalInput").ap()
    out_d = nc.dram_tensor("out", [O, T, V], f32, kind="ExternalOutput").ap()

    qa_d = nc.dram_tensor("qa_scr", [H, F_PAD], f32).ap()
    qw_d = nc.dram_tensor("qw_scr", [H, GL], bf16).ap()
    ka_d = nc.dram_tensor("ka_scr", [H, GL], f32).ap()
    kw_d = nc.dram_tensor("kw_scr", [H, GL], bf16).ap()
    row_d = nc.dram_tensor("row_scr", [6, C], f32).ap()

    with tile.TileContext(nc) as tc:
        with (
            tc.tile_pool(name="consts", bufs=1) as cp,
            tc.tile_pool(name="data", bufs=1) as dp,
        ):
            # ---------- input load first (weights go on other DMA queues) ----
            p1x_cm = tc.tile_pool(name="p1_x", bufs=1)
            p1x = p1x_cm.__enter__()
            x_f = [p1x.tile([128, F_PAD], f32, tag=f"x_f{hh}", name=f"x_f{hh}")
                   for hh in range(2)]
            for hh in range(2):
                nc.gpsimd.dma_start(out=_r(x_f[hh][:, V:F - V]),
                                    in_=_r(x_d[hh * 128:(hh + 1) * 128, :, :]))

            def load_bias_col(nm):
                t = cp.tile([128, 2], f32, tag=f"b_{nm}", name=f"b_{nm}")
                src = bass.AP(tensor=wd[nm].tensor, offset=wd[nm].offset,
                              ap=[[1, 128], [128, 2]])
                nc.scalar.dma_start(out=t, in_=src)
                return t

            bias = {nm: load_bias_col(nm) for nm in bnames}

            eps_t = cp.tile([128, 1], f32, tag="eps", name="eps_t")
            nc.vector.memset(eps_t, EPS)

            def fill_r(t, value):
                nc.scalar.activation(out=_r(t), in_=_r(t), func=AF.Copy,
                                     bias=float(value), scale=0.0)

            onesC = cp.tile([128, 128], f32, tag="onesC", name="onesC")
            fill_r(onesC, 1.0 / C)
            onesC_b = cp.tile([128, 128], bf16, tag="onesC_b", name="onesC_b")
            nc.scalar.activation(out=onesC_b, in_=onesC, func=AF.Copy)

            # ---- bf16 stationaries (ln1_g folded into Wq/Wk/Wv/Wqa rows) ----
            wqb = [cp.tile([128, C], bf16, tag=f"wqb{kh}", name=f"wqb{kh}") for kh in range(2)]
            wkb = [cp.tile([128, C], bf16, tag=f"wkb{kh}", name=f"wkb{kh}") for kh in range(2)]
            wvb = [cp.tile([128, C], bf16, tag=f"wvb{kh}", name=f"wvb{kh}") for kh in range(2)]
            wpb = [cp.tile([128, C], bf16, tag=f"wpb{kh}", name=f"wpb{kh}") for kh in range(2)]
            w2b = [cp.tile([128, C], bf16, tag=f"w2b{kh}", name=f"w2b{kh}") for kh in range(2)]
            w1g = [cp.tile([128, C], bf16, tag=f"w1g{kh}", name=f"w1g{kh}") for kh in range(2)]
            c1g = [cp.tile([128, C], bf16, tag=f"c1g{kh}", name=f"c1g{kh}") for kh in range(2)]
            wqab = [cp.tile([128, H], bf16, tag=f"wqab{kh}", name=f"wqab{kh}") for kh in range(2)]
            wkab = [cp.tile([128, H], bf16, tag=f"wkab{kh}", name=f"wkab{kh}") for kh in range(2)]
            c2t = []
            for w in range(W):
                c2t.append([cp.tile([128, O], bf16, tag=f"w_c2_{w}{kh}", name=f"w_c2_{w}{kh}")
                            for kh in range(2)])
            wtpb = [cp.tile([128, C], bf16, tag=f"wtpb{kh}", name=f"wtpb{kh}") for kh in range(2)]
            ident_b = cp.tile([128, 128], bf16, tag="ident_b", name="ident_b")

            # ---------- setup-scoped: Wtp = Wt@Wp, bias rows, c2 cast ----------
            with (
                tc.tile_pool(name="setup_sb", bufs=1) as sp,
                tc.tile_pool(name="setup_ps", bufs=2, space="PSUM") as spp,
            ):
                # raw f32 weights (setup-scoped; freed before phase 1)
                wt = {}
                for i, nm in enumerate(["Wq", "Wk", "Wv", "Wp", "W1", "W2", "c1_w"]):
                    wt[nm] = [sp.tile([128, C], f32, tag=f"w_{nm}{kh}", name=f"w_{nm}{kh}")
                              for kh in range(2)]
                    eng = nc.scalar if i % 2 == 0 else nc.sync
                    for kh in range(2):
                        eng.dma_start(out=_r(wt[nm][kh]),
                                      in_=_r(wd[nm][kh * 128:(kh + 1) * 128, :]))
                for nm in ["Wqa", "Wka"]:
                    wt[nm] = [sp.tile([128, H], f32, tag=f"w_{nm}{kh}", name=f"w_{nm}{kh}")
                              for kh in range(2)]
                    for kh in range(2):
                        nc.scalar.dma_start(out=_r(wt[nm][kh]),
                                            in_=_r(wd[nm][kh * 128:(kh + 1) * 128, :]))
                # ln1_g folded into Wq/Wk/Wv rows; wqab = g * (Wq @ Wqa)
                # below — the reference pools attention logits from q
                # (= nx@Wq + bq); the bq/bqa shifts cancel under softmax.
                for kh in range(2):
                    g_col = bias["ln1_g"][:, kh:kh + 1]
                    nc.vector.tensor_scalar_mul(wqb[kh], wt["Wq"][kh], g_col)
                    nc.vector.tensor_scalar_mul(wkb[kh], wt["Wk"][kh], g_col)
                    nc.vector.tensor_scalar_mul(wvb[kh], wt["Wv"][kh], g_col)
                    nc.vector.tensor_scalar_mul(w1g[kh], wt["W1"][kh],
                                                bias["ffn_g"][:, kh:kh + 1])
                    nc.vector.tensor_scalar_mul(c1g[kh], wt["c1_w"][kh],
                                                bias["tn_g"][:, kh:kh + 1])
                    nc.scalar.activation(out=wpb[kh], in_=wt["Wp"][kh], func=AF.Copy)
                    nc.scalar.activation(out=w2b[kh], in_=wt["W2"][kh], func=AF.Copy)
                    nc.vector.tensor_scalar_mul(wkab[kh], wt["Wka"][kh], 1.0)

                c2f = sp.tile([128, O], f32, tag="c2f", bufs=2, name="c2f")
                for w in range(W):
                    for kh in range(2):
                        c2f_ = sp.tile([128, O], f32, tag="c2f", bufs=2, name="c2f_")
                        nc.sync.dma_start(out=c2f_,
                                          in_=wd["c2_w"][w, kh * 128:(kh + 1) * 128, :])
                        nc.vector.tensor_copy(c2t[w][kh], c2f_)

                wtw = [sp.tile([128, C], f32, tag=f"wt{kh}", name=f"wtw{kh}")
                       for kh in range(2)]
                for kh in range(2):
                    nc.sync.dma_start(out=wtw[kh],
                                      in_=wd["Wt"][kh * 128:(kh + 1) * 128, :])
                ident = sp.tile([128, 128], f32, tag="ident", name="ident")
                make_identity(nc, ident)
                nc.scalar.activation(out=ident_b, in_=ident, func=AF.Copy)

                for kh in range(2):
                    pacc = spp.tile([128, C], f32, tag="wtp_acc", name="pacc")
                    pqa_w = spp.tile([128, H], f32, tag="qae_acc", name="pqa_w")
                    for mh in range(2):
                        ptr = spp.tile([128, 128], f32, tag="tr", name="ptr")
                        nc.tensor.transpose(ptr, wtw[kh][:, mh * 128:(mh + 1) * 128], ident)
                        a_t = sp.tile([128, 128], f32, tag="a_t", name="a_t")
                        nc.scalar.activation(out=_r(a_t), in_=ptr, func=AF.Copy)
                        nc.tensor.matmul(pacc, _r(a_t), _r(wt["Wp"][mh]),
                                         start=(mh == 0), stop=(mh == 1))
                        # Wqa_eff[kh] = sum_m Wq[kh rows, m]^T.T @ Wqa[m]
                        ptr2 = spp.tile([128, 128], f32, tag="tr", name="ptr2")
                        nc.tensor.transpose(ptr2, wt["Wq"][kh][:, mh * 128:(mh + 1) * 128], ident)
                        a_t2 = sp.tile([128, 128], f32, tag="a_t", name="a_t2")
                        nc.scalar.activation(out=_r(a_t2), in_=ptr2, func=AF.Copy)
                        nc.tensor.matmul(pqa_w, _r(a_t2), _r(wt["Wqa"][mh]),
                                         start=(mh == 0), stop=(mh == 1))
                    nc.scalar.activation(out=wtpb[kh], in_=pacc, func=AF.Copy)
                    nc.vector.tensor_scalar_mul(wqab[kh], pqa_w,
                                                bias["ln1_g"][:, kh:kh + 1])

                def colvec(nm, kh):
                    t = sp.tile([128, 1], f32, tag="cv", bufs=4, name=f"cv_{nm}{kh}")
                    src = bass.AP(tensor=wd[nm].tensor, offset=wd[nm].offset + kh * 128,
                                  ap=[[1, 128], [128, 1]])
                    nc.sync.dma_start(out=_r(t), in_=_r(src))
                    return t

                def rowvec(nm):
                    t = sp.tile([1, C], f32, tag="rv", bufs=4, name=f"rv_{nm}")
                    nc.sync.dma_start(out=t, in_=wd[nm])
                    return t

                # rows: btp = bt@Wp + bp; B1 = ffn_b@W1 + b1; Bc1 = tn_b@c1_w
                #       + c1_b; bq' = ln1_b@Wq + bq; similarly bk', bv'
                for i, (bnm, wmat, addnm) in enumerate([
                    ("bt", wt["Wp"], "bp"),
                    ("ffn_b", wt["W1"], "b1"),
                    ("tn_b", wt["c1_w"], "c1_b"),
                    ("ln1_b", wt["Wq"], "bq"),
                    ("ln1_b", wt["Wk"], "bk"),
                    ("ln1_b", wt["Wv"], "bv"),
                ]):
                    pr = spp.tile([1, C], f32, tag="rowacc", name="pr")
                    for kh in range(2):
                        nc.tensor.matmul(pr, _r(colvec(bnm, kh)), _r(wmat[kh]),
                                         start=(kh == 0), stop=(kh == 1))
                    row_i = sp.tile([1, C], f32, tag="row_i", bufs=3, name=f"row_i{i}")
                    nc.vector.tensor_add(row_i, pr, rowvec(addnm))
                    nc.sync.dma_start(out=row_d[i:i + 1, :], in_=row_i)

            # bounce bias rows back into per-partition [128, 2] layout
            btp_t = cp.tile([128, 2], f32, tag="btp", name="btp_t")
            B1_t = cp.tile([128, 2], f32, tag="B1", name="B1_t")
            Bc1_t = cp.tile([128, 2], f32, tag="Bc1", name="Bc1_t")
            bq_t = cp.tile([128, 2], f32, tag="bqf", name="bq_t")
            bk_t = cp.tile([128, 2], f32, tag="bkf", name="bk_t")
            bv_t = cp.tile([128, 2], f32, tag="bvf", name="bv_t")
            for i, t in enumerate([btp_t, B1_t, Bc1_t, bq_t, bk_t, bv_t]):
                for kh in range(2):
                    src = bass.AP(tensor=row_d.tensor,
                                  offset=row_d.offset + i * C + kh * 128,
                                  ap=[[1, 128], [128, 1]])
                    nc.sync.dma_start(out=t[:, kh:kh + 1], in_=src)

            # ---------- persistent activations ----------
            q_f = [dp.tile([128, F_PAD], bf16, tag=f"q_f{hh}", name=f"q_f{hh}") for hh in range(2)]
            k_f = [dp.tile([128, F_PAD], bf16, tag=f"k_f{hh}", name=f"k_f{hh}") for hh in range(2)]
            v_f = [dp.tile([128, F_PAD], bf16, tag=f"v_f{hh}", name=f"v_f{hh}") for hh in range(2)]
            px_f = [dp.tile([128, F_PAD], bf16, tag=f"px_f{hh}", name=f"px_f{hh}") for hh in range(2)]
            pq_b = [dp.tile([128, G], bf16, tag=f"pqb{hh}", name=f"pqb{hh}") for hh in range(2)]
            pk_b = [dp.tile([128, G], bf16, tag=f"pkb{hh}", name=f"pkb{hh}") for hh in range(2)]

            def softmax_chunk(src_gather_ap, dst_dram, g0, pool, tagp):
                """Softmax over L per (group, head) in [128 = 16g x 8h, L]
                layout; writes normalized bf16 weights to dst_dram."""
                ag = pool.tile([128, L], f32, tag="sm_ag", bufs=4, name=f"ag_{tagp}")
                nc.gpsimd.dma_start(out=ag, in_=src_gather_ap)
                mx = pool.tile([128, 1], f32, tag="sm_mx", bufs=4, name=f"mx_{tagp}")
                nc.vector.reduce_max(mx, ag, axis=AX.X)
                e = pool.tile([128, L], f32, tag="sm_e", bufs=4, name=f"e_{tagp}")
                nc.vector.tensor_scalar_sub(e, ag, mx[:, 0:1])
                nc.scalar.activation(out=e, in_=e, func=AF.Exp, scale=SCALE)
                sm = pool.tile([128, 1], f32, tag="sm_s", bufs=4, name=f"sm_{tagp}")
                nc.vector.reduce_sum(sm, e, axis=AX.X)
                rs = pool.tile([128, 1], f32, tag="sm_rs", bufs=4, name=f"rs_{tagp}")
                nc.vector.reciprocal(rs, sm)
                wgn = pool.tile([128, L], bf16, tag="sm_w", bufs=4, name=f"wgn_{tagp}")
                nc.vector.tensor_scalar_mul(wgn, e, rs[:, 0:1])
                dst = bass.AP(tensor=dst_dram.tensor,
                              offset=dst_dram.offset + g0 * L,
                              ap=[[L, CH_G], [GL, H], [1, L]])
                nc.gpsimd.dma_start(out=dst, in_=wgn)

            # ================= phase 1 + front =================
            fp_cm = tc.tile_pool(name="front_sb", bufs=1)
            fp = fp_cm.__enter__()
            qw_bc = [fp.tile([128, GL], bf16, tag="bc", bufs=2, name=f"qwbc{hh}")
                     for hh in range(2)]

            p1_cm = tc.tile_pool(name="p1_sb", bufs=2)
            pp1_cm = tc.tile_pool(name="p1_ps", bufs=1, space="PSUM")
            pp1m_cm = tc.tile_pool(name="p1_mm", bufs=4, space="PSUM")
            p1 = p1_cm.__enter__()
            pp1 = pp1_cm.__enter__()
            pp1m = pp1m_cm.__enter__()
            if True:
                for hh in range(2):
                    fill_r(x_f[hh][:, 0:V], 0.0)
                    fill_r(x_f[hh][:, F - V:F_PAD], 0.0)

                def phase1_fsub(s):
                    sl = slice(s * FSUB, (s + 1) * FSUB)
                    x2 = [p1.tile([128, FSUB], f32, tag=f"x2_{hh}", name=f"x2_{hh}")
                          for hh in range(2)]
                    for hh in range(2):
                        nc.vector.scalar_tensor_tensor(
                            out=_r(x2[hh]), in0=x_f[hh][:, sl], scalar=1.0,
                            in1=x_f[hh][:, sl], op0=ALU.mult, op1=ALU.mult)
                    pmean = pp1.tile([128, FSUB], f32, tag="pmean", name="pmean")
                    pmsq = pp1.tile([128, FSUB], f32, tag="pmsq", name="pmsq")
                    for hh in range(2):
                        nc.tensor.matmul(pmean, _r(onesC), _r(x_f[hh][:, sl]),
                                         start=(hh == 0), stop=(hh == 1))
                    for hh in range(2):
                        nc.tensor.matmul(pmsq, _r(onesC), _r(x2[hh]),
                                         start=(hh == 0), stop=(hh == 1))
                    m2 = p1.tile([128, FSUB], f32, tag="m2", name="m2")
                    nc.scalar.activation(out=m2, in_=pmean, func=AF.Square)
                    var = p1.tile([128, FSUB], f32, tag="var", name="var")
                    nc.vector.tensor_sub(var, pmsq, m2)
                    sd = p1.tile([128, FSUB], f32, tag="sd", name="sd")
                    nc.scalar.activation(out=sd, in_=var, func=AF.Sqrt, bias=eps_t)
                    rstd = p1.tile([128, FSUB], f32, tag="rstd", name="rstd")
                    nc.vector.reciprocal_approx_fast(out=rstd, in_=sd)
                    nx = []
                    for hh in range(2):
                        xc = p1.tile([128, FSUB], f32, tag=f"xc{hh}", name=f"xc{hh}")
                        nc.vector.tensor_sub(xc, x_f[hh][:, sl], pmean)
                        nxh = p1.tile([128, FSUB], bf16, tag=f"nx{hh}", name=f"nx{hh}")
                        nc.vector.tensor_mul(nxh, xc, rstd)
                        nx.append(nxh)
                    for nm_w, b_t, dst, eng in [(wqb, bq_t, q_f, "s"),
                                                (wkb, bk_t, k_f, "s"),
                                                (wvb, bv_t, v_f, "v")]:
                        for mh in range(2):
                            pm_ = pp1m.tile([128, FSUB], f32, tag="mm", name="pm_")
                            for kh in range(2):
                                nc.tensor.matmul(pm_,
                                                 nm_w[kh][:, mh * 128:(mh + 1) * 128],
                                                 nx[kh], start=(kh == 0), stop=(kh == 1))
                            if eng == "s":
                                nc.scalar.activation(out=dst[mh][:, sl], in_=pm_,
                                                     func=AF.Identity,
                                                     bias=b_t[:, mh:mh + 1])
                            else:
                                nc.vector.tensor_scalar_add(dst[mh][:, sl], pm_,
                                                            b_t[:, mh:mh + 1])
                    pqa = pp1.tile([H, FSUB], f32, tag="pqa", name="pqa")
                    for kh in range(2):
                        nc.tensor.matmul(pqa, wqab[kh], nx[kh],
                                         start=(kh == 0), stop=(kh == 1))
                    qa_s = p1.tile([H, FSUB], f32, tag="qa_s", bufs=3, name="qa_s")
                    nc.vector.tensor_copy(qa_s, pqa)
                    nc.sync.dma_start(out=qa_d[:, sl], in_=qa_s)

                def qw_chunk(cc):
                    g0 = cc * CH_G
                    src = bass.AP(tensor=qa_d.tensor, offset=qa_d.offset + g0 * V,
                                  ap=[[V, CH_G], [F_PAD, H], [1, L]])
                    softmax_chunk(src, qw_d, g0, p1, "q")
                    # broadcast this chunk of qw into group-stage layout
                    for hh in range(2):
                        src_b = bass.AP(
                            tensor=qw_d.tensor,
                            offset=qw_d.offset + (hh * 4) * GL + g0 * L,
                            ap=[[GL, 4], [0, 32], [1, CH_G * L]])
                        nc.sync.dma_start(out=qw_bc[hh][:, g0 * L:(g0 + CH_G) * L],
                                          in_=src_b)

                for s in range(N_FSUB):
                    phase1_fsub(s)
                # qw chunks emitted after phase 1; gathers/softmax execute
                # early anyway (dep-gated), and the Exps batch in one
                # exp-table era
                for cc in range(N_CH):
                    qw_chunk(cc)

                # ---- pooled query (per-chunk product + reduce) ----
                pq_t = [dp.tile([128, G], f32, tag=f"pq{hh}", name=f"pq{hh}")
                        for hh in range(2)]
                for cc in range(N_CH):
                    g0 = cc * CH_G
                    gsl = slice(g0, g0 + CH_G)
                    csl = slice(g0 * L, (g0 + CH_G) * L)
                    for hh in range(2):
                        prod = fp.tile([128, CH_G * L], bf16, tag="prod", bufs=2,
                                       name=f"prodq{hh}")
                        nc.vector.scalar_tensor_tensor(
                            out=prod, in0=unf(q_f[hh], g0, CH_G), scalar=1.0,
                            in1=qw_bc[hh][:, csl], op0=ALU.mult, op1=ALU.mult)
                        nc.vector.reduce_sum(pq_t[hh][:, gsl],
                                             _view(prod, 0, [[L, CH_G], [1, L]]),
                                             axis=AX.X)
                        nc.vector.tensor_copy(pq_b[hh][:, gsl], pq_t[hh][:, gsl])

                # ---- px = Wp@q + btp + x (deferred: fills PE gap here) ----
                for s in range(N_FSUB):
                    sl = slice(s * FSUB, (s + 1) * FSUB)
                    for mh in range(2):
                        pp_ = pp1m.tile([128, FSUB], f32, tag="mm", name="pp_")
                        for kh in range(2):
                            nc.tensor.matmul(pp_, wpb[kh][:, mh * 128:(mh + 1) * 128],
                                             q_f[kh][:, sl], start=(kh == 0),
                                             stop=(kh == 1))
                        nc.vector.scalar_tensor_tensor(
                            out=px_f[mh][:, sl], in0=pp_, scalar=btp_t[:, mh:mh + 1],
                            in1=x_f[mh][:, sl], op0=ALU.add, op1=ALU.add)

                pp1m_cm.__exit__(None, None, None)
                pp1_cm.__exit__(None, None, None)

                # ---- kp + ka matmuls, kw softmax + pooled key pk
                # interleaved per covered chunk (flowing pipeline) ----
                kw_bc = [fp.tile([128, GL], bf16, tag="bc", bufs=2,
                                 name=f"kwbc{hh}") for hh in range(2)]
                pk_t = [dp.tile([128, G], f32, tag=f"pk{hh}", name=f"pk{hh}")
                        for hh in range(2)]
                kw_after = {2: 0, 5: 1, 7: 2, 10: 3, 13: 4, 15: 5, 18: 6, 21: 7}

                def kw_chunk(cc):
                    g0 = cc * CH_G
                    gsl = slice(g0, g0 + CH_G)
                    csl = slice(g0 * L, (g0 + CH_G) * L)
                    src = bass.AP(tensor=ka_d.tensor,
                                  offset=ka_d.offset + g0 * L,
                                  ap=[[L, CH_G], [GL, H], [1, L]])
                    softmax_chunk(src, kw_d, g0, p1, "k")
                    for hh in range(2):
                        src_b = bass.AP(
                            tensor=kw_d.tensor,
                            offset=kw_d.offset + (hh * 4) * GL + g0 * L,
                            ap=[[GL, 4], [0, 32], [1, CH_G * L]])
                        nc.sync.dma_start(out=kw_bc[hh][:, csl], in_=src_b)
                    for hh in range(2):
                        prod = fp.tile([128, CH_G * L], bf16, tag="prod", bufs=2,
                                       name=f"prodk{hh}")
                        nc.vector.scalar_tensor_tensor(
                            out=prod, in0=unf(k_f[hh], g0, CH_G), scalar=1.0,
                            in1=kw_bc[hh][:, csl], op0=ALU.mult, op1=ALU.mult)
                        nc.vector.reduce_sum(pk_t[hh][:, gsl],
                                             _view(prod, 0, [[L, CH_G], [1, L]]),
                                             axis=AX.X)
                        nc.vector.tensor_copy(pk_b[hh][:, gsl], pk_t[hh][:, gsl])

                with tc.tile_pool(name="ka_ps", bufs=2, space="PSUM") as kap:
                    for b, (ga, gc) in enumerate(KA_BLOCKS):
                        cw = gc * L
                        col0 = ga * L
                        kp_blk = []
                        for hh in range(2):
                            kpb = fp.tile([128, 6 * L], bf16, tag="kp", bufs=4,
                                          name=f"kp{hh}")
                            eng = nc.gpsimd if (b + hh) % 2 == 0 else nc.vector
                            eng.tensor_tensor(
                                out=_view(kpb, 0, [[L, gc], [1, L]]),
                                in0=unf(k_f[hh], ga, gc),
                                in1=bc_g(pq_b[hh], ga, gc), op=ALU.mult)
                            kp_blk.append(kpb)
                        pka = kap.tile([H, 6 * L], f32, tag="ka", name="pka")
                        for hh in range(2):
                            nc.tensor.matmul(pka[:, 0:cw], wkab[hh],
                                             kp_blk[hh][:, 0:cw],
                                             start=(hh == 0), stop=(hh == 1))
                        ka_s = fp.tile([H, 6 * L], f32, tag="ka_s", bufs=3,
                                       name="ka_s")
                        nc.scalar.activation(out=ka_s[:, 0:cw],
                                             in_=pka[:, 0:cw], func=AF.Copy)
                        nc.scalar.dma_start(out=ka_d[:, col0:col0 + cw],
                                            in_=ka_s[:, 0:cw])
                        if b in kw_after:
                            kw_chunk(kw_after[b])

            p1_cm.__exit__(None, None, None)
            fp_cm.__exit__(None, None, None)
            p1x_cm.__exit__(None, None, None)

            # ================= backs: 2 halves x 11 blocks =================
            with (
                tc.tile_pool(name="bk_sb", bufs=1) as bp,
                tc.tile_pool(name="bk_sm", bufs=1) as bs,
                tc.tile_pool(name="bk_ps", bufs=1, space="PSUM") as bps,
            ):
                def big(name):
                    return bp.tile([128, HALF_COLS], bf16, tag="big", bufs=8,
                                   name=name)

                def small(tag, dt=bf16):
                    return bs.tile([128, 6 * L], dt, tag=tag, bufs=4, name=tag)

                def layer_mm(pm, wpair, rhs_pair, cols):
                    for mh in range(2):
                        for kh in range(2):
                            nc.tensor.matmul(pm[mh], wpair[kh][:, mh * 128:(mh + 1) * 128],
                                             rhs_pair[kh][:, cols],
                                             start=(kh == 0), stop=(kh == 1))

                for ih, g0h in enumerate([0, GH]):
                    # one "big" tag, bufs=8: call i+8 reuses call i's buffer.
                    # Order guarantees the prior tenant's last-read stage
                    # strictly precedes the new tenant's first-write stage
                    # (pool slot reuse is tile-granular).
                    z_t = [big(f"z{ih}{hh}") for hh in range(2)]
                    px_u = [big(f"px{ih}{mh}") for mh in range(2)]
                    xr1 = [big(f"xr{ih}{mh}") for mh in range(2)]
                    att_b = [big(f"att{ih}{mh}") for mh in range(2)]
                    g1 = [big(f"g1{ih}{mh}") for mh in range(2)]
                    y_b = [big(f"y{ih}{mh}") for mh in range(2)]
                    yr = [big(f"yr{ih}{mh}") for mh in range(2)]
                    h_t = [big(f"h{ih}{mh}") for mh in range(2)]

                    # --- z = v * pk (per block, alternating gpsimd/vector) ---
                    for hh in range(2):
                        for bi, (ga, gc) in enumerate(BLOCKS):
                            eng = nc.gpsimd if (bi + hh) % 2 == 0 else nc.vector
                            eng.tensor_tensor(
                                out=_view(z_t[hh], ga * L, [[L, gc], [1, L]]),
                                in0=unf(v_f[hh], g0h + ga, gc),
                                in1=bc_g(pk_b[hh], g0h + ga, gc), op=ALU.mult)
                    # --- px_unf via SBUF->SBUF window DMA (3 slabs) ---
                    for mh in range(2):
                        for (ga, gc) in [(0, 22), (22, 21), (43, 21)]:
                            nc.sync.dma_start(
                                out=px_u[mh][:, ga * L:(ga + gc) * L],
                                in_=unf(px_f[mh], g0h + ga, gc))

                    def fused_ln_block(pm, res_pair, b2col, dst_pair, xr_pair,
                                       cols, cw):
                        """dst = pm + res (+b2), sq = dst^2, LN stats,
                        xr = (dst - mean) * rstd. Per-block."""
                        for mh in range(2):
                            nc.vector.scalar_tensor_tensor(
                                out=dst_pair[mh][:, cols], in0=pm[mh],
                                scalar=(1.0 if b2col is None
                                        else b2col[:, mh:mh + 1]),
                                in1=res_pair[mh][:, cols],
                                op0=(ALU.mult if b2col is None else ALU.add),
                                op1=ALU.add)
                        sqs = []
                        for mh in range(2):
                            sq = small(f"sq{mh}")
                            nc.vector.tensor_tensor(
                                out=sq[:, 0:cw], in0=dst_pair[mh][:, cols],
                                in1=dst_pair[mh][:, cols], op=ALU.mult)
                            sqs.append(sq)
                        mps = bps.tile([128, 6 * L], f32, tag="mean", bufs=2,
                                       name="mps")[:, 0:cw]
                        sps = bps.tile([128, 6 * L], f32, tag="msq", bufs=2,
                                       name="sps")[:, 0:cw]
                        for mh in range(2):
                            nc.tensor.matmul(mps, onesC_b, dst_pair[mh][:, cols],
                                             start=(mh == 0), stop=(mh == 1))
                        for mh in range(2):
                            nc.tensor.matmul(sps, onesC_b, sqs[mh][:, 0:cw],
                                             start=(mh == 0), stop=(mh == 1))
                        meanb = small("meanb")
                        nc.scalar.activation(out=meanb[:, 0:cw], in_=mps,
                                             func=AF.Copy)
                        m2 = small("m2")
                        nc.vector.tensor_tensor(out=m2[:, 0:cw],
                                                in0=meanb[:, 0:cw],
                                                in1=meanb[:, 0:cw], op=ALU.mult)
                        var = small("var", f32)
                        nc.vector.scalar_tensor_tensor(
                            out=var[:, 0:cw], in0=sps, scalar=1.0, in1=m2[:, 0:cw],
                            op0=ALU.mult, op1=ALU.subtract)
                        sd = small("sd", f32)
                        nc.scalar.activation(out=sd[:, 0:cw], in_=var[:, 0:cw],
                                             func=AF.Sqrt, bias=eps_t)
                        rstd = small("rstd", f32)
                        nc.vector.reciprocal_approx_fast(out=rstd[:, 0:cw],
                                                         in_=sd[:, 0:cw])
                        for mh in range(2):
                            xc = small(f"xc{mh}")
                            nc.vector.tensor_sub(xc[:, 0:cw], dst_pair[mh][:, cols],
                                                 meanb[:, 0:cw])
                            nc.vector.tensor_mul(xr_pair[mh][:, cols], xc[:, 0:cw],
                                                 rstd[:, 0:cw])

                    # --- stage P: att = Wtp@z + px, LN fold -> xr1 ---
                    for (ga, gc) in BLOCKS:
                        cols = slice(ga * L, (ga + gc) * L)
                        cw = gc * L
                        pm = [bps.tile([128, 6 * L], f32, tag="mm", bufs=2,
                                       name=f"pmP{mh}")[:, 0:cw] for mh in range(2)]
                        layer_mm(pm, wtpb, z_t, cols)
                        fused_ln_block(pm, px_u, None, att_b, xr1, cols, cw)

                    # --- W1 -> gelu ---
                    for (ga, gc) in BLOCKS:
                        cols = slice(ga * L, (ga + gc) * L)
                        cw = gc * L
                        pm = [bps.tile([128, 6 * L], f32, tag="mm", bufs=2,
                                       name=f"pm1{mh}")[:, 0:cw] for mh in range(2)]
                        layer_mm(pm, w1g, xr1, cols)
                        for mh in range(2):
                            nc.scalar.activation(out=g1[mh][:, cols], in_=pm[mh],
                                                 func=AF.Gelu,
                                                 bias=B1_t[:, mh:mh + 1])

                    # --- W2: y = W2@g1 + b2 + att, LN fold -> yr ---
                    for (ga, gc) in BLOCKS:
                        cols = slice(ga * L, (ga + gc) * L)
                        cw = gc * L
                        pm = [bps.tile([128, 6 * L], f32, tag="mm", bufs=2,
                                       name=f"pm2{mh}")[:, 0:cw] for mh in range(2)]
                        layer_mm(pm, w2b, g1, cols)
                        fused_ln_block(pm, att_b, bias["b2"], y_b, yr, cols, cw)

                    # --- c1 -> gelu, h in w-major layout [128, W, GH*V] ---
                    for (ga, gc) in BLOCKS:
                        cols = slice(ga * L, (ga + gc) * L)
                        pm = [bps.tile([128, 6 * L], f32, tag="mm", bufs=2,
                                       name=f"pm3{mh}")[:, 0:gc * L] for mh in range(2)]
                        layer_mm(pm, c1g, yr, cols)
                        for mh in range(2):
                            dst = _view(h_t[mh], ga * V,
                                        [[V, gc], [GH * V, W], [1, V]])
                            src = _view(pm[mh], 0, [[L, gc], [V, W], [1, V]])
                            nc.scalar.activation(out=dst, in_=src, func=AF.Gelu,
                                                 bias=Bc1_t[:, mh:mh + 1])

                    # --- c2: contract (w, kh) -> out [O, GH*V] ---
                    for mh in range(2):
                        for cb in range(N_C2B):
                            po = bps.tile([128, C2B], f32, tag="po", bufs=2,
                                          name="po")
                            first = True
                            for w in range(W):
                                for kh in range(2):
                                    c0 = w * GH * V + cb * C2B
                                    nc.tensor.matmul(
                                        po, c2t[w][kh][:, mh * 128:(mh + 1) * 128],
                                        h_t[kh][:, c0:c0 + C2B], start=first,
                                        stop=(w == W - 1 and kh == 1))
                                    first = False
                            os_ = bs.tile([128, C2B], f32, tag="os", bufs=3,
                                          name="os_")
                            nc.scalar.activation(out=os_, in_=po, func=AF.Identity,
                                                 bias=bias["c2_b"][:, mh:mh + 1])
                            nc.sync.dma_start(
                                out=out_d[mh * 128:(mh + 1) * 128,
                                          g0h + cb * 16:g0h + (cb + 1) * 16, :],
                                in_=os_)
    return nc


_CACHE = {}


def _get_compiled():
    if "nc" not in _CACHE:
        nc = bacc.Bacc("TRN2", target_bir_lowering=False, debug=False)
        build(nc)
        nc.compile()
        _CACHE["nc"] = nc
    return _CACHE["nc"]


def kernel(**inputs):
    nc = _get_compiled()
    x = np.asarray(inputs["x"], dtype=np.float32)
    n = x.shape[0]
    names = ["Wq", "Wk", "Wv", "Wt", "Wp", "W1", "W2", "c1_w", "Wqa", "Wka",
             "c2_w", "ln1_g", "ln1_b", "bq", "bk", "bv", "bt", "bp", "ffn_g",
             "ffn_b", "b1", "b2", "tn_g", "tn_b", "c1_b", "c2_b"]
    shared = {nm: np.asarray(inputs[nm], dtype=np.float32) for nm in names}
    in_maps = [{"x": x[i], **shared} for i in range(n)]
    res = bass_utils.run_bass_kernel_spmd(nc, in_maps, core_ids=list(range(n)))
    return np.stack([res.results[i]["out"] for i in range(n)], axis=0)


if __name__ == "__main__":
    nc = bacc.Bacc("TRN2", target_bir_lowering=False, debug=False)
    build(nc)
    nc.compile()
    print("build+compile OK")


# revision 32
# speedup vs baseline: 1.1683x; 1.0110x over previous
"""Trainium2 Bass kernel for nn_Block_47193100648803.

Contract: kernel(**inputs) takes FULL unsharded inputs (numpy), returns the
FULL (N, O, T, V) output. Internally shards data-parallel over N across the
8 NeuronCores (one batch element per core, weights replicated).

v2 design (from trace analysis of v1 @ 709us):
- Channels on SBUF partitions (C=256 -> 2 half tiles), tokens on free axis.
- Phase 1 (frame domain, 3328 cols): LN1 (PE ones-matmul stats), q/k/v/qa
  projections in bf16. px = Wp@q + btp + x deferred to fill the PE gap
  during the softmax-heavy front.
- Front (global, group domain 9600 cols): qw softmax in 8 chunks (gather
  from DRAM bounce), pooled query pq via product+reduce, kp per-block on
  GpSimd, ka matmuls, kw softmax, pooled key pk. All Exp work stays in the
  natural_log_exp activation-table era (no table thrash).
- Backs (2 halves x 11 blocks, stage-major): att = Wtp@z + I@px_unf (PE
  identity-accumulated residual), LN folds with explicit mean subtraction
  (vector, bf16 2x mode), FFN + temporal conv. Scalar activation stream is
  batched per stage so table set switches drop from ~70 to ~10.
- Softmax biases bqa/bka are dropped entirely (softmax shift invariance);
  ln1 gamma/beta folded into the QKV weights/biases at setup.
"""

import os
import sys

import numpy as np

for _p in ("/opt/trn_rl_repo", "/root/.axon_site/_ro/trn_rl_repo"):
    if os.path.isdir(_p) and _p not in sys.path:
        sys.path.append(_p)

import concourse.bass as bass
import concourse.tile as tile
from concourse import bacc, bass_utils, mybir
from concourse.masks import make_identity

f32 = mybir.dt.float32
f32r = mybir.dt.float32r
bf16 = mybir.dt.bfloat16
AF = mybir.ActivationFunctionType
ALU = mybir.AluOpType
AX = mybir.AxisListType

# ---- problem constants (hardcoded per spec) ----
N_CORES = 8
C, T, V = 256, 128, 25
H = 8
W = 3
O = 256
L = W * V                 # 75
FT = T + 2                # 130 padded frames
F = FT * V                # 3250 real frame columns (zero pads at both ends)
F_PAD = 3328              # allocated frame columns
G = T                     # 128 groups per core
GL = G * L                # 9600 group-stage columns
SCALE = 1.0 / (32.0 ** 0.5)
EPS = 1e-5

FSUB = 416                # phase-1 matmul column tile
N_FSUB = F_PAD // FSUB    # 8
CH_G = 16                 # groups per softmax chunk (16*8 heads = 128 parts)
N_CH = G // CH_G          # 8

GH = 64                   # groups per back half
HALF_COLS = GH * L        # 4800
# back blocks within a half: 10x6 groups + 1x4 groups (450/300 cols)
BLOCKS = [(i * 6, 6) for i in range(10)] + [(60, 4)]
# global 6-group blocks for kp/ka (21x450 + 1x150)
KA_BLOCKS = [(i * 6, 6) for i in range(21)] + [(126, 2)]
C2B = 400                 # c2 output block (16 groups * 25)
N_C2B = HALF_COLS // 3 // C2B  # 1600/400 = 4


def _r(ap):
    return ap.bitcast(f32r)


def _view(t, offset, dims):
    """AP view on tile t: partition dim kept, free dims replaced."""
    return bass.AP(tensor=t.tensor, offset=t.offset + offset, ap=[t.ap[0]] + dims)


def unf(t, g0, gc):
    """Overlapping window view [128, gc, W, V] on a [128, F] frame tile."""
    return _view(t, g0 * V, [[V, gc], [V, W], [1, V]])


def bc_g(t, g0, gc):
    """Broadcast per-(c,g) [128, G] tile over L -> [128, gc, L] (step-0)."""
    return _view(t, g0, [[1, gc], [0, L]])


def build(nc):
    x_d = nc.dram_tensor("x", [C, T, V], f32, kind="ExternalInput").ap()
    wd = {}
    for nm in ["Wq", "Wk", "Wv", "Wt", "Wp", "W1", "W2", "c1_w"]:
        wd[nm] = nc.dram_tensor(nm, [C, C], f32, kind="ExternalInput").ap()
    wd["Wqa"] = nc.dram_tensor("Wqa", [C, H], f32, kind="ExternalInput").ap()
    wd["Wka"] = nc.dram_tensor("Wka", [C, H], f32, kind="ExternalInput").ap()
    wd["c2_w"] = nc.dram_tensor("c2_w", [W, C, O], f32, kind="ExternalInput").ap()
    bnames = ["ln1_g", "ln1_b", "bq", "bk", "bv", "bt", "bp", "ffn_g", "ffn_b",
              "b1", "b2", "tn_g", "tn_b", "c1_b", "c2_b"]
    for nm in bnames:
        wd[nm] = nc.dram_tensor(nm, [C], f32, kind="ExternalInput").ap()
    out_d = nc.dram_tensor("out", [O, T, V], f32, kind="ExternalOutput").ap()

    qa_d = nc.dram_tensor("qa_scr", [H, F_PAD], f32).ap()
    qw_d = nc.dram_tensor("qw_scr", [H, GL], bf16).ap()
    ka_d = nc.dram_tensor("ka_scr", [H, GL], f32).ap()
    kw_d = nc.dram_tensor("kw_scr", [H, GL], bf16).ap()
    row_d = nc.dram_tensor("row_scr", [6, C], f32).ap()

    with tile.TileContext(nc) as tc:
        with (
            tc.tile_pool(name="consts", bufs=1) as cp,
            tc.tile_pool(name="data", bufs=1) as dp,
        ):
            # ---------- input load first (weights go on other DMA queues) ----
            fp_cm = tc.tile_pool(name="front_sb", bufs=1)
            fp = fp_cm.__enter__()
            p1x_cm = tc.tile_pool(name="p1_x", bufs=1)
            p1x = p1x_cm.__enter__()
            x_f = [p1x.tile([128, F_PAD], f32, tag=f"x_f{hh}", name=f"x_f{hh}")
                   for hh in range(2)]
            for hh in range(2):
                nc.gpsimd.dma_start(out=_r(x_f[hh][:, V:F - V]),
                                    in_=_r(x_d[hh * 128:(hh + 1) * 128, :, :]))

            def load_bias_col(nm):
                t = cp.tile([128, 2], f32, tag=f"b_{nm}", name=f"b_{nm}")
                src = bass.AP(tensor=wd[nm].tensor, offset=wd[nm].offset,
                              ap=[[1, 128], [128, 2]])
                nc.scalar.dma_start(out=t, in_=src)
                return t

            bias = {nm: load_bias_col(nm) for nm in bnames}

            eps_t = cp.tile([128, 1], f32, tag="eps", name="eps_t")
            nc.vector.memset(eps_t, EPS)

            def fill_r(t, value):
                nc.scalar.activation(out=_r(t), in_=_r(t), func=AF.Copy,
                                     bias=float(value), scale=0.0)

            onesC = cp.tile([128, 128], f32, tag="onesC", name="onesC")
            fill_r(onesC, 1.0 / C)
            onesC_b = cp.tile([128, 128], bf16, tag="onesC_b", name="onesC_b")
            nc.scalar.activation(out=onesC_b, in_=onesC, func=AF.Copy)

            # ---- bf16 stationaries (ln1_g folded into Wq/Wk/Wv/Wqa rows) ----
            wqb = [cp.tile([128, C], bf16, tag=f"wqb{kh}", name=f"wqb{kh}") for kh in range(2)]
            wkb = [cp.tile([128, C], bf16, tag=f"wkb{kh}", name=f"wkb{kh}") for kh in range(2)]
            wvb = [cp.tile([128, C], bf16, tag=f"wvb{kh}", name=f"wvb{kh}") for kh in range(2)]
            wpb = [cp.tile([128, C], bf16, tag=f"wpb{kh}", name=f"wpb{kh}") for kh in range(2)]
            w2b = [cp.tile([128, C], bf16, tag=f"w2b{kh}", name=f"w2b{kh}") for kh in range(2)]
            w1g = [cp.tile([128, C], bf16, tag=f"w1g{kh}", name=f"w1g{kh}") for kh in range(2)]
            c1g = [cp.tile([128, C], bf16, tag=f"c1g{kh}", name=f"c1g{kh}") for kh in range(2)]
            wqab = [cp.tile([128, H], bf16, tag=f"wqab{kh}", name=f"wqab{kh}") for kh in range(2)]
            wkab = [cp.tile([128, H], bf16, tag=f"wkab{kh}", name=f"wkab{kh}") for kh in range(2)]
            c2t = []
            for w in range(W):
                c2t.append([cp.tile([128, O], bf16, tag=f"w_c2_{w}{kh}", name=f"w_c2_{w}{kh}")
                            for kh in range(2)])
            wtpb = [cp.tile([128, C], bf16, tag=f"wtpb{kh}", name=f"wtpb{kh}") for kh in range(2)]
            ident_b = cp.tile([128, 128], bf16, tag="ident_b", name="ident_b")

            # ---------- setup-scoped: Wtp = Wt@Wp, bias rows, c2 cast ----------
            with (
                tc.tile_pool(name="setup_sb", bufs=1) as sp,
                tc.tile_pool(name="setup_ps", bufs=2, space="PSUM") as spp,
            ):
                # raw f32 weights (setup-scoped; freed before phase 1)
                wt = {}
                for i, nm in enumerate(["Wq", "Wk", "Wv", "Wp", "W1", "W2", "c1_w"]):
                    wt[nm] = [sp.tile([128, C], f32, tag=f"w_{nm}{kh}", name=f"w_{nm}{kh}")
                              for kh in range(2)]
                    eng = nc.scalar if i % 2 == 0 else nc.sync
                    for kh in range(2):
                        eng.dma_start(out=_r(wt[nm][kh]),
                                      in_=_r(wd[nm][kh * 128:(kh + 1) * 128, :]))
                for nm in ["Wqa", "Wka"]:
                    wt[nm] = [sp.tile([128, H], f32, tag=f"w_{nm}{kh}", name=f"w_{nm}{kh}")
                              for kh in range(2)]
                    for kh in range(2):
                        nc.scalar.dma_start(out=_r(wt[nm][kh]),
                                            in_=_r(wd[nm][kh * 128:(kh + 1) * 128, :]))
                # ln1_g folded into Wq/Wk/Wv rows; wqab = g * (Wq @ Wqa)
                # below — the reference pools attention logits from q
                # (= nx@Wq + bq); the bq/bqa shifts cancel under softmax.
                for kh in range(2):
                    g_col = bias["ln1_g"][:, kh:kh + 1]
                    nc.vector.tensor_scalar_mul(wqb[kh], wt["Wq"][kh], g_col)
                    nc.vector.tensor_scalar_mul(wkb[kh], wt["Wk"][kh], g_col)
                    nc.vector.tensor_scalar_mul(wvb[kh], wt["Wv"][kh], g_col)
                    nc.vector.tensor_scalar_mul(w1g[kh], wt["W1"][kh],
                                                bias["ffn_g"][:, kh:kh + 1])
                    nc.vector.tensor_scalar_mul(c1g[kh], wt["c1_w"][kh],
                                                bias["tn_g"][:, kh:kh + 1])
                    nc.scalar.activation(out=wpb[kh], in_=wt["Wp"][kh], func=AF.Copy)
                    nc.scalar.activation(out=w2b[kh], in_=wt["W2"][kh], func=AF.Copy)
                    nc.vector.tensor_scalar_mul(wkab[kh], wt["Wka"][kh], 1.0)

                c2f = sp.tile([128, O], f32, tag="c2f", bufs=2, name="c2f")
                for w in range(W):
                    for kh in range(2):
                        c2f_ = sp.tile([128, O], f32, tag="c2f", bufs=2, name="c2f_")
                        nc.sync.dma_start(out=c2f_,
                                          in_=wd["c2_w"][w, kh * 128:(kh + 1) * 128, :])
                        nc.vector.tensor_copy(c2t[w][kh], c2f_)

                wtw = [sp.tile([128, C], f32, tag=f"wt{kh}", name=f"wtw{kh}")
                       for kh in range(2)]
                for kh in range(2):
                    nc.sync.dma_start(out=wtw[kh],
                                      in_=wd["Wt"][kh * 128:(kh + 1) * 128, :])
                ident = sp.tile([128, 128], f32, tag="ident", name="ident")
                make_identity(nc, ident)
                nc.scalar.activation(out=ident_b, in_=ident, func=AF.Copy)

                for kh in range(2):
                    pacc = spp.tile([128, C], f32, tag="wtp_acc", name="pacc")
                    pqa_w = spp.tile([128, H], f32, tag="qae_acc", name="pqa_w")
                    for mh in range(2):
                        ptr = spp.tile([128, 128], f32, tag="tr", name="ptr")
                        nc.tensor.transpose(ptr, wtw[kh][:, mh * 128:(mh + 1) * 128], ident)
                        a_t = sp.tile([128, 128], f32, tag="a_t", name="a_t")
                        nc.scalar.activation(out=_r(a_t), in_=ptr, func=AF.Copy)
                        nc.tensor.matmul(pacc, _r(a_t), _r(wt["Wp"][mh]),
                                         start=(mh == 0), stop=(mh == 1))
                        # Wqa_eff[kh] = sum_m Wq[kh rows, m]^T.T @ Wqa[m]
                        ptr2 = spp.tile([128, 128], f32, tag="tr", name="ptr2")
                        nc.tensor.transpose(ptr2, wt["Wq"][kh][:, mh * 128:(mh + 1) * 128], ident)
                        a_t2 = sp.tile([128, 128], f32, tag="a_t", name="a_t2")
                        nc.scalar.activation(out=_r(a_t2), in_=ptr2, func=AF.Copy)
                        nc.tensor.matmul(pqa_w, _r(a_t2), _r(wt["Wqa"][mh]),
                                         start=(mh == 0), stop=(mh == 1))
                    nc.scalar.activation(out=wtpb[kh], in_=pacc, func=AF.Copy)
                    nc.vector.tensor_scalar_mul(wqab[kh], pqa_w,
                                                bias["ln1_g"][:, kh:kh + 1])

                def colvec(nm, kh):
                    t = sp.tile([128, 1], f32, tag="cv", bufs=4, name=f"cv_{nm}{kh}")
                    src = bass.AP(tensor=wd[nm].tensor, offset=wd[nm].offset + kh * 128,
                                  ap=[[1, 128], [128, 1]])
                    nc.sync.dma_start(out=_r(t), in_=_r(src))
                    return t

                def rowvec(nm):
                    t = sp.tile([1, C], f32, tag="rv", bufs=4, name=f"rv_{nm}")
                    nc.sync.dma_start(out=t, in_=wd[nm])
                    return t

                # rows: btp = bt@Wp + bp; B1 = ffn_b@W1 + b1; Bc1 = tn_b@c1_w
                #       + c1_b; bq' = ln1_b@Wq + bq; similarly bk', bv'
                for i, (bnm, wmat, addnm) in enumerate([
                    ("bt", wt["Wp"], "bp"),
                    ("ffn_b", wt["W1"], "b1"),
                    ("tn_b", wt["c1_w"], "c1_b"),
                    ("ln1_b", wt["Wq"], "bq"),
                    ("ln1_b", wt["Wk"], "bk"),
                    ("ln1_b", wt["Wv"], "bv"),
                ]):
                    pr = spp.tile([1, C], f32, tag="rowacc", name="pr")
                    for kh in range(2):
                        nc.tensor.matmul(pr, _r(colvec(bnm, kh)), _r(wmat[kh]),
                                         start=(kh == 0), stop=(kh == 1))
                    row_i = sp.tile([1, C], f32, tag="row_i", bufs=3, name=f"row_i{i}")
                    nc.vector.tensor_add(row_i, pr, rowvec(addnm))
                    nc.sync.dma_start(out=row_d[i:i + 1, :], in_=row_i)

            # bounce bias rows back into per-partition [128, 2] layout
            btp_t = cp.tile([128, 2], f32, tag="btp", name="btp_t")
            B1_t = cp.tile([128, 2], f32, tag="B1", name="B1_t")
            Bc1_t = cp.tile([128, 2], f32, tag="Bc1", name="Bc1_t")
            bq_t = cp.tile([128, 2], f32, tag="bqf", name="bq_t")
            bk_t = cp.tile([128, 2], f32, tag="bkf", name="bk_t")
            bv_t = cp.tile([128, 2], f32, tag="bvf", name="bv_t")
            for i, t in enumerate([btp_t, B1_t, Bc1_t, bq_t, bk_t, bv_t]):
                for kh in range(2):
                    src = bass.AP(tensor=row_d.tensor,
                                  offset=row_d.offset + i * C + kh * 128,
                                  ap=[[1, 128], [128, 1]])
                    nc.sync.dma_start(out=t[:, kh:kh + 1], in_=src)

            # ---------- persistent activations ----------
            q_f = [p1x.tile([128, F_PAD], bf16, tag=f"q_f{hh}", name=f"q_f{hh}") for hh in range(2)]
            k_f = [dp.tile([128, F_PAD], bf16, tag=f"k_f{hh}", name=f"k_f{hh}") for hh in range(2)]
            v_f = [dp.tile([128, F_PAD], bf16, tag=f"v_f{hh}", name=f"v_f{hh}") for hh in range(2)]
            px_f = [dp.tile([128, F_PAD], bf16, tag=f"px_f{hh}", name=f"px_f{hh}") for hh in range(2)]
            pq_b = [dp.tile([128, G], bf16, tag=f"pqb{hh}", name=f"pqb{hh}") for hh in range(2)]
            pk_b = [dp.tile([128, G], bf16, tag=f"pkb{hh}", name=f"pkb{hh}") for hh in range(2)]

            def softmax_chunk(src_gather_ap, dst_dram, g0, pool, tagp):
                """Softmax over L per (group, head) in [128 = 16g x 8h, L]
                layout; writes normalized bf16 weights to dst_dram."""
                ag = pool.tile([128, L], f32, tag="sm_ag", bufs=2, name=f"ag_{tagp}")
                nc.gpsimd.dma_start(out=ag, in_=src_gather_ap)
                mx = pool.tile([128, 1], f32, tag="sm_mx", bufs=2, name=f"mx_{tagp}")
                nc.vector.reduce_max(mx, ag, axis=AX.X)
                e = pool.tile([128, L], f32, tag="sm_e", bufs=2, name=f"e_{tagp}")
                nc.vector.tensor_scalar_sub(e, ag, mx[:, 0:1])
                nc.scalar.activation(out=e, in_=e, func=AF.Exp, scale=SCALE)
                sm = pool.tile([128, 1], f32, tag="sm_s", bufs=2, name=f"sm_{tagp}")
                nc.vector.reduce_sum(sm, e, axis=AX.X)
                rs = pool.tile([128, 1], f32, tag="sm_rs", bufs=2, name=f"rs_{tagp}")
                nc.vector.reciprocal(rs, sm)
                wgn = pool.tile([128, L], bf16, tag="sm_w", bufs=2, name=f"wgn_{tagp}")
                nc.vector.tensor_scalar_mul(wgn, e, rs[:, 0:1])
                dst = bass.AP(tensor=dst_dram.tensor,
                              offset=dst_dram.offset + g0 * L,
                              ap=[[L, CH_G], [GL, H], [1, L]])
                nc.gpsimd.dma_start(out=dst, in_=wgn)

            # ================= phase 1 + front =================
            qw_bc = [fp.tile([128, GL], bf16, tag="bc", bufs=2, name=f"qwbc{hh}")
                     for hh in range(2)]

            p1_cm = tc.tile_pool(name="p1_sb", bufs=2)
            pp1_cm = tc.tile_pool(name="p1_ps", bufs=1, space="PSUM")
            pp1m_cm = tc.tile_pool(name="p1_mm", bufs=4, space="PSUM")
            p1 = p1_cm.__enter__()
            pp1 = pp1_cm.__enter__()
            pp1m = pp1m_cm.__enter__()
            if True:
                for hh in range(2):
                    fill_r(x_f[hh][:, 0:V], 0.0)
                    fill_r(x_f[hh][:, F - V:F_PAD], 0.0)

                def phase1_fsub(s):
                    sl = slice(s * FSUB, (s + 1) * FSUB)
                    x2 = [p1.tile([128, FSUB], f32, tag=f"x2_{hh}", name=f"x2_{hh}")
                          for hh in range(2)]
                    for hh in range(2):
                        nc.scalar.activation(out=_r(x2[hh]), in_=x_f[hh][:, sl],
                                             func=AF.Square)
                    pmean = pp1.tile([128, FSUB], f32, tag="pmean", name="pmean")
                    pmsq = pp1.tile([128, FSUB], f32, tag="pmsq", name="pmsq")
                    for hh in range(2):
                        nc.tensor.matmul(pmean, _r(onesC), _r(x_f[hh][:, sl]),
                                         start=(hh == 0), stop=(hh == 1))
                    for hh in range(2):
                        nc.tensor.matmul(pmsq, _r(onesC), _r(x2[hh]),
                                         start=(hh == 0), stop=(hh == 1))
                    m2 = p1.tile([128, FSUB], f32, tag="m2", name="m2")
                    nc.scalar.activation(out=m2, in_=pmean, func=AF.Square)
                    var = p1.tile([128, FSUB], f32, tag="var", name="var")
                    nc.vector.tensor_sub(var, pmsq, m2)
                    sd = p1.tile([128, FSUB], f32, tag="sd", name="sd")
                    nc.scalar.activation(out=sd, in_=var, func=AF.Sqrt, bias=eps_t)
                    rstd = p1.tile([128, FSUB], f32, tag="rstd", name="rstd")
                    nc.vector.reciprocal_approx_fast(out=rstd, in_=sd)
                    nx = []
                    for hh in range(2):
                        xc = p1.tile([128, FSUB], f32, tag=f"xc{hh}", name=f"xc{hh}")
                        nc.vector.tensor_sub(xc, x_f[hh][:, sl], pmean)
                        nxh = p1.tile([128, FSUB], bf16, tag=f"nx{hh}", name=f"nx{hh}")
                        nc.vector.tensor_mul(nxh, xc, rstd)
                        nx.append(nxh)
                    for nm_w, b_t, dst, eng in [(wqb, bq_t, q_f, "s"),
                                                (wkb, bk_t, k_f, "v"),
                                                (wvb, bv_t, v_f, "s")]:
                        for mh in range(2):
                            pm_ = pp1m.tile([128, FSUB], f32, tag="mm", name="pm_")
                            for kh in range(2):
                                nc.tensor.matmul(pm_,
                                                 nm_w[kh][:, mh * 128:(mh + 1) * 128],
                                                 nx[kh], start=(kh == 0), stop=(kh == 1))
                            if eng == "s":
                                nc.scalar.activation(out=dst[mh][:, sl], in_=pm_,
                                                     func=AF.Identity,
                                                     bias=b_t[:, mh:mh + 1])
                            else:
                                nc.vector.tensor_scalar_add(dst[mh][:, sl], pm_,
                                                            b_t[:, mh:mh + 1])
                    pqa = pp1.tile([H, FSUB], f32, tag="pqa", name="pqa")
                    for kh in range(2):
                        nc.tensor.matmul(pqa, wqab[kh], nx[kh],
                                         start=(kh == 0), stop=(kh == 1))
                    qa_s = p1.tile([H, FSUB], f32, tag="qa_s", bufs=3, name="qa_s")
                    nc.scalar.activation(out=qa_s, in_=pqa, func=AF.Copy)
                    nc.sync.dma_start(out=qa_d[:, sl], in_=qa_s)

                def qw_chunk(cc):
                    g0 = cc * CH_G
                    src = bass.AP(tensor=qa_d.tensor, offset=qa_d.offset + g0 * V,
                                  ap=[[V, CH_G], [F_PAD, H], [1, L]])
                    softmax_chunk(src, qw_d, g0, fp, "q")
                    # broadcast this chunk of qw into group-stage layout
                    for hh in range(2):
                        src_b = bass.AP(
                            tensor=qw_d.tensor,
                            offset=qw_d.offset + (hh * 4) * GL + g0 * L,
                            ap=[[GL, 4], [0, 32], [1, CH_G * L]])
                        nc.sync.dma_start(out=qw_bc[hh][:, g0 * L:(g0 + CH_G) * L],
                                          in_=src_b)

                for s in range(N_FSUB):
                    phase1_fsub(s)
                # qw chunks emitted after phase 1; gathers/softmax execute
                # early anyway (dep-gated), and the Exps batch in one
                # exp-table era
                for cc in range(N_CH):
                    qw_chunk(cc)

                # ---- pooled query (per-chunk product + reduce) ----
                pq_t = [dp.tile([128, G], f32, tag=f"pq{hh}", name=f"pq{hh}")
                        for hh in range(2)]
                for cc in range(N_CH):
                    g0 = cc * CH_G
                    gsl = slice(g0, g0 + CH_G)
                    csl = slice(g0 * L, (g0 + CH_G) * L)
                    for hh in range(2):
                        prod = fp.tile([128, CH_G * L], bf16, tag="prod", bufs=2,
                                       name=f"prodq{hh}")
                        nc.vector.scalar_tensor_tensor(
                            out=prod, in0=unf(q_f[hh], g0, CH_G), scalar=1.0,
                            in1=qw_bc[hh][:, csl], op0=ALU.mult, op1=ALU.mult)
                        nc.vector.reduce_sum(pq_t[hh][:, gsl],
                                             _view(prod, 0, [[L, CH_G], [1, L]]),
                                             axis=AX.X)
                        nc.vector.tensor_copy(pq_b[hh][:, gsl], pq_t[hh][:, gsl])

                # ---- px = Wp@q + btp + x (deferred: fills PE gap here) ----
                for s in range(N_FSUB):
                    sl = slice(s * FSUB, (s + 1) * FSUB)
                    for mh in range(2):
                        pp_ = pp1m.tile([128, FSUB], f32, tag="mm", name="pp_")
                        for kh in range(2):
                            nc.tensor.matmul(pp_, wpb[kh][:, mh * 128:(mh + 1) * 128],
                                             q_f[kh][:, sl], start=(kh == 0),
                                             stop=(kh == 1))
                        nc.vector.scalar_tensor_tensor(
                            out=px_f[mh][:, sl], in0=pp_, scalar=btp_t[:, mh:mh + 1],
                            in1=x_f[mh][:, sl], op0=ALU.add, op1=ALU.add)

            pp1m_cm.__exit__(None, None, None)
            pp1_cm.__exit__(None, None, None)
            p1_cm.__exit__(None, None, None)
            p1x_cm.__exit__(None, None, None)

            # ---- kp (gpsimd) + ka matmuls, kw softmax + pooled key pk,
            # interleaved per covered chunk; split around back half 0 so the
            # PE queue is never head-blocked on late kp blocks ----
            kw_bc = [fp.tile([128, GL], bf16, tag="bc", bufs=2,
                             name=f"kwbc{hh}") for hh in range(2)]
            pk_t = [dp.tile([128, G], f32, tag=f"pk{hh}", name=f"pk{hh}")
                    for hh in range(2)]

            def kw_chunk(cc):
                g0 = cc * CH_G
                gsl = slice(g0, g0 + CH_G)
                csl = slice(g0 * L, (g0 + CH_G) * L)
                src = bass.AP(tensor=ka_d.tensor,
                              offset=ka_d.offset + g0 * L,
                              ap=[[L, CH_G], [GL, H], [1, L]])
                softmax_chunk(src, kw_d, g0, fp, "k")
                for hh in range(2):
                    src_b = bass.AP(
                        tensor=kw_d.tensor,
                        offset=kw_d.offset + (hh * 4) * GL + g0 * L,
                        ap=[[GL, 4], [0, 32], [1, CH_G * L]])
                    nc.scalar.dma_start(out=kw_bc[hh][:, csl], in_=src_b)
                for hh in range(2):
                    prod = fp.tile([128, CH_G * L], bf16, tag="prod", bufs=2,
                                   name=f"prodk{hh}")
                    nc.vector.scalar_tensor_tensor(
                        out=prod, in0=unf(k_f[hh], g0, CH_G), scalar=1.0,
                        in1=kw_bc[hh][:, csl], op0=ALU.mult, op1=ALU.mult)
                    nc.vector.reduce_sum(pk_t[hh][:, gsl],
                                         _view(prod, 0, [[L, CH_G], [1, L]]),
                                         axis=AX.X)
                    nc.vector.tensor_copy(pk_b[hh][:, gsl], pk_t[hh][:, gsl])

            kap_cm = tc.tile_pool(name="ka_ps", bufs=2, space="PSUM")
            kap = kap_cm.__enter__()

            def ka_blocks(blo, bhi, kw_after):
                for b in range(blo, bhi):
                    ga, gc = KA_BLOCKS[b]
                    cw = gc * L
                    col0 = ga * L
                    kp_blk = []
                    for hh in range(2):
                        kpb = fp.tile([128, 6 * L], bf16, tag="kp", bufs=4,
                                      name=f"kp{hh}")
                        nc.gpsimd.tensor_tensor(
                            out=_view(kpb, 0, [[L, gc], [1, L]]),
                            in0=unf(k_f[hh], ga, gc),
                            in1=bc_g(pq_b[hh], ga, gc), op=ALU.mult)
                        kp_blk.append(kpb)
                    pka = kap.tile([H, 6 * L], f32, tag="ka", name="pka")
                    for hh in range(2):
                        nc.tensor.matmul(pka[:, 0:cw], wkab[hh],
                                         kp_blk[hh][:, 0:cw],
                                         start=(hh == 0), stop=(hh == 1))
                    ka_s = fp.tile([H, 6 * L], f32, tag="ka_s", bufs=2,
                                   name="ka_s")
                    nc.scalar.activation(out=ka_s[:, 0:cw],
                                         in_=pka[:, 0:cw], func=AF.Copy)
                    nc.scalar.dma_start(out=ka_d[:, col0:col0 + cw],
                                        in_=ka_s[:, 0:cw])
                    if b in kw_after:
                        kw_chunk(kw_after[b])

            ka_blocks(0, 11, {2: 0, 5: 1, 7: 2, 10: 3})

            # ================= backs: 2 halves x 11 blocks =================
            with (
                tc.tile_pool(name="bk_sb", bufs=1) as bp,
                tc.tile_pool(name="bk_sm", bufs=1) as bs,
                tc.tile_pool(name="bk_ps", bufs=1, space="PSUM") as bps,
            ):
                def big(name):
                    return bp.tile([128, HALF_COLS], bf16, tag="big", bufs=8,
                                   name=name)

                def small(tag, dt=bf16):
                    return bs.tile([128, 6 * L], dt, tag=tag, bufs=2, name=tag)

                def layer_mm(pm, wpair, rhs_pair, cols):
                    for mh in range(2):
                        for kh in range(2):
                            nc.tensor.matmul(pm[mh], wpair[kh][:, mh * 128:(mh + 1) * 128],
                                             rhs_pair[kh][:, cols],
                                             start=(kh == 0), stop=(kh == 1))

                def back_half(ih, g0h):
                    # one "big" tag, bufs=8: call i+8 reuses call i's buffer.
                    # Order guarantees the prior tenant's last-read stage
                    # strictly precedes the new tenant's first-write stage
                    # (pool slot reuse is tile-granular).
                    z_t = [big(f"z{ih}{hh}") for hh in range(2)]
                    px_u = [big(f"px{ih}{mh}") for mh in range(2)]
                    xr1 = [big(f"xr{ih}{mh}") for mh in range(2)]
                    att_b = [big(f"att{ih}{mh}") for mh in range(2)]
                    g1 = [big(f"g1{ih}{mh}") for mh in range(2)]
                    y_b = [big(f"y{ih}{mh}") for mh in range(2)]
                    yr = [big(f"yr{ih}{mh}") for mh in range(2)]
                    h_t = [big(f"h{ih}{mh}") for mh in range(2)]

                    # --- z = v * pk (per block, gpsimd) ---
                    for hh in range(2):
                        for (ga, gc) in BLOCKS:
                            nc.gpsimd.tensor_tensor(
                                out=_view(z_t[hh], ga * L, [[L, gc], [1, L]]),
                                in0=unf(v_f[hh], g0h + ga, gc),
                                in1=bc_g(pk_b[hh], g0h + ga, gc), op=ALU.mult)
                    # --- px_unf via SBUF->SBUF window DMA (3 slabs) ---
                    for mh in range(2):
                        for (ga, gc) in [(0, 22), (22, 21), (43, 21)]:
                            nc.sync.dma_start(
                                out=px_u[mh][:, ga * L:(ga + gc) * L],
                                in_=unf(px_f[mh], g0h + ga, gc))

                    def fused_ln_block(pm, res_pair, b2col, dst_pair, xr_pair,
                                       cols, cw):
                        """dst = pm + res (+b2), sq = dst^2, LN stats,
                        xr = (dst - mean) * rstd. Per-block."""
                        for mh in range(2):
                            nc.vector.scalar_tensor_tensor(
                                out=dst_pair[mh][:, cols], in0=pm[mh],
                                scalar=(1.0 if b2col is None
                                        else b2col[:, mh:mh + 1]),
                                in1=res_pair[mh][:, cols],
                                op0=(ALU.mult if b2col is None else ALU.add),
                                op1=ALU.add)
                        sqs = []
                        for mh in range(2):
                            sq = small(f"sq{mh}")
                            nc.scalar.activation(out=sq[:, 0:cw],
                                                 in_=dst_pair[mh][:, cols],
                                                 func=AF.Square)
                            sqs.append(sq)
                        mps = bps.tile([128, 6 * L], f32, tag="mean", bufs=1,
                                       name="mps")[:, 0:cw]
                        sps = bps.tile([128, 6 * L], f32, tag="msq", bufs=1,
                                       name="sps")[:, 0:cw]
                        for mh in range(2):
                            nc.tensor.matmul(mps, onesC_b, dst_pair[mh][:, cols],
                                             start=(mh == 0), stop=(mh == 1))
                        for mh in range(2):
                            nc.tensor.matmul(sps, onesC_b, sqs[mh][:, 0:cw],
                                             start=(mh == 0), stop=(mh == 1))
                        meanb = small("meanb")
                        nc.scalar.activation(out=meanb[:, 0:cw], in_=mps,
                                             func=AF.Copy)
                        m2 = small("m2")
                        nc.gpsimd.tensor_tensor(out=m2[:, 0:cw],
                                                in0=meanb[:, 0:cw],
                                                in1=meanb[:, 0:cw], op=ALU.mult)
                        var = small("var", f32)
                        nc.vector.scalar_tensor_tensor(
                            out=var[:, 0:cw], in0=sps, scalar=1.0, in1=m2[:, 0:cw],
                            op0=ALU.mult, op1=ALU.subtract)
                        sd = small("sd", f32)
                        nc.scalar.activation(out=sd[:, 0:cw], in_=var[:, 0:cw],
                                             func=AF.Sqrt, bias=eps_t)
                        rstd = small("rstd", f32)
                        nc.vector.reciprocal_approx_fast(out=rstd[:, 0:cw],
                                                         in_=sd[:, 0:cw])
                        for mh in range(2):
                            xc = small(f"xc{mh}")
                            nc.gpsimd.tensor_sub(xc[:, 0:cw], dst_pair[mh][:, cols],
                                                 meanb[:, 0:cw])
                            nc.vector.tensor_mul(xr_pair[mh][:, cols], xc[:, 0:cw],
                                                 rstd[:, 0:cw])

                    # --- stage P: att = Wtp@z + px, LN fold -> xr1 ---
                    for (ga, gc) in BLOCKS:
                        cols = slice(ga * L, (ga + gc) * L)
                        cw = gc * L
                        pm = [bps.tile([128, 6 * L], f32, tag="mm", bufs=2,
                                       name=f"pmP{mh}")[:, 0:cw] for mh in range(2)]
                        layer_mm(pm, wtpb, z_t, cols)
                        fused_ln_block(pm, px_u, None, att_b, xr1, cols, cw)

                    # --- W1 -> gelu ---
                    for (ga, gc) in BLOCKS:
                        cols = slice(ga * L, (ga + gc) * L)
                        cw = gc * L
                        pm = [bps.tile([128, 6 * L], f32, tag="mm", bufs=2,
                                       name=f"pm1{mh}")[:, 0:cw] for mh in range(2)]
                        layer_mm(pm, w1g, xr1, cols)
                        for mh in range(2):
                            nc.scalar.activation(out=g1[mh][:, cols], in_=pm[mh],
                                                 func=AF.Gelu,
                                                 bias=B1_t[:, mh:mh + 1])

                    # --- W2: y = W2@g1 + b2 + att, LN fold -> yr ---
                    for (ga, gc) in BLOCKS:
                        cols = slice(ga * L, (ga + gc) * L)
                        cw = gc * L
                        pm = [bps.tile([128, 6 * L], f32, tag="mm", bufs=2,
                                       name=f"pm2{mh}")[:, 0:cw] for mh in range(2)]
                        layer_mm(pm, w2b, g1, cols)
                        fused_ln_block(pm, att_b, bias["b2"], y_b, yr, cols, cw)

                    # --- c1 -> gelu, h in w-major layout [128, W, GH*V] ---
                    for (ga, gc) in BLOCKS:
                        cols = slice(ga * L, (ga + gc) * L)
                        pm = [bps.tile([128, 6 * L], f32, tag="mm", bufs=2,
                                       name=f"pm3{mh}")[:, 0:gc * L] for mh in range(2)]
                        layer_mm(pm, c1g, yr, cols)
                        for mh in range(2):
                            dst = _view(h_t[mh], ga * V,
                                        [[V, gc], [GH * V, W], [1, V]])
                            src = _view(pm[mh], 0, [[L, gc], [V, W], [1, V]])
                            nc.scalar.activation(out=dst, in_=src, func=AF.Gelu,
                                                 bias=Bc1_t[:, mh:mh + 1])

                    # --- c2: contract (w, kh) -> out [O, GH*V] ---
                    for mh in range(2):
                        for cb in range(N_C2B):
                            po = bps.tile([128, C2B], f32, tag="po", bufs=2,
                                          name="po")
                            first = True
                            for w in range(W):
                                for kh in range(2):
                                    c0 = w * GH * V + cb * C2B
                                    nc.tensor.matmul(
                                        po, c2t[w][kh][:, mh * 128:(mh + 1) * 128],
                                        h_t[kh][:, c0:c0 + C2B], start=first,
                                        stop=(w == W - 1 and kh == 1))
                                    first = False
                            os_ = bs.tile([128, C2B], f32, tag="os", bufs=2,
                                          name="os_")
                            nc.scalar.activation(out=os_, in_=po, func=AF.Identity,
                                                 bias=bias["c2_b"][:, mh:mh + 1])
                            nc.sync.dma_start(
                                out=out_d[mh * 128:(mh + 1) * 128,
                                          g0h + cb * 16:g0h + (cb + 1) * 16, :],
                                in_=os_)

                back_half(0, 0)
                ka_blocks(11, 22, {13: 4, 15: 5, 18: 6, 21: 7})
                back_half(1, GH)

            kap_cm.__exit__(None, None, None)
            fp_cm.__exit__(None, None, None)
    return nc


_CACHE = {}


def _get_compiled():
    if "nc" not in _CACHE:
        nc = bacc.Bacc("TRN2", target_bir_lowering=False, debug=False)
        build(nc)
        nc.compile()
        _CACHE["nc"] = nc
    return _CACHE["nc"]


def kernel(**inputs):
    nc = _get_compiled()
    x = np.asarray(inputs["x"], dtype=np.float32)
    n = x.shape[0]
    names = ["Wq", "Wk", "Wv", "Wt", "Wp", "W1", "W2", "c1_w", "Wqa", "Wka",
             "c2_w", "ln1_g", "ln1_b", "bq", "bk", "bv", "bt", "bp", "ffn_g",
             "ffn_b", "b1", "b2", "tn_g", "tn_b", "c1_b", "c2_b"]
    shared = {nm: np.asarray(inputs[nm], dtype=np.float32) for nm in names}
    in_maps = [{"x": x[i], **shared} for i in range(n)]
    res = bass_utils.run_bass_kernel_spmd(nc, in_maps, core_ids=list(range(n)))
    return np.stack([res.results[i]["out"] for i in range(n)], axis=0)


if __name__ == "__main__":
    nc = bacc.Bacc("TRN2", target_bir_lowering=False, debug=False)
    build(nc)
    nc.compile()
    print("build+compile OK")
